# revision 63
# baseline (speedup 1.0000x reference)
"""Trainium2 Bass kernel for CodeAttention (B=4, S=2048, E=768, H=12).

Sharding: 8 cores = 4 batches x 2 head-groups (6 heads each). Each core
computes a partial projection output for its batch; the host sums the two
partials per batch and adds the (host-folded) bias row.

Design (fp16 datapath; ~149us/core cost-model estimate vs the 270us
fused baseline; max rel err ~6.7e-4):
- Key compaction: the padding mask is known on the host, so masked keys
  (~50%) are gathered OUT of the K/V stream entirely (exact math: they
  contribute to neither the numerator nor the softmax denominator). Kept
  keys are padded to KP (multiple of 128) with zero columns whose ones-
  column entry is 0, which keeps them exactly inert.
- pv orientation flip: out[q,65] = sum_k pt[k,q]*vst[k,65] makes the
  moving operand the 65-wide V tile, cutting pv PE rows ~2x vs moving
  the 512-wide query block. The 65th column accumulates the softmax
  denominator, so normalization is a per-partition reciprocal+scale on
  DVE (no gpsimd broadcast). Two query-chunk accumulators share each
  PSUM bank under a single start/stop group.
- x arrives pre-transposed from the host (xt, xtk), so there are no
  on-chip x transposes; att is re-transposed on PE (48 small transposes)
  for the output projection, and y leaves as yT (host re-transposes).
- Main rhythm: per (query-half, head) 9 key-chunk slots: exp(kc) on ACT,
  then scores(kc+1) (one slot of lookahead keeps ACT fed -- cross-engine
  deps are emission-ordered counters), then pv groups of the previous
  head, then statically scheduled filler units (q/k/v/proj/attT) spread
  so no head period overloads PE (v is head-pair granular).
- The modeled DMA bus is near serial: all input loads ride one queue in
  exact priority order (first-exp critical path leads); the first two
  exps are split into 512-wide halves so ACT starts before xt-sb1 lands.
- Last head: its pv pairs accumulate DURING its own slots (un ring +
  drained pv ring), so the tail is norms -> attT dt2 -> output
  projections, with the final stores batched into 3-tile DMAs.
"""

import sys

if "/opt/trn_rl_repo" not in sys.path:
    sys.path.insert(0, "/opt/trn_rl_repo")

import numpy as np

import concourse.bass as bass  # noqa: F401
import concourse.mybir as mybir
import concourse.tile as tile
from concourse import bacc
from concourse.alu_op_type import AluOpType
from concourse.bass_utils import run_bass_kernel_spmd
from concourse.masks import make_identity

F32 = mybir.dt.float32
F32R = mybir.dt.float32r
FP16 = mybir.dt.float16
Act = mybir.ActivationFunctionType

B, S, E, H, D = 4, 2048, 768, 12, 64
HC = 6                    # heads per core
KCH = E // 128            # contraction chunks over E = 6
VC = HC * D               # v columns per core = 384
VW = D + 1                # v width incl. ones column = 65
DEFAULT_KP = 1152         # padded kept-key count for the fixed-seed mask


def build_program(kp=DEFAULT_KP):
    nkc = kp // 128
    nc = bacc.Bacc("TRN2", target_bir_lowering=False, debug=False, num_devices=8)

    xt_d = nc.dram_tensor("xt", [4, 128, KCH, 512], FP16, kind="ExternalInput")
    xtk_d = nc.dram_tensor("xtk", [128, KCH, kp], FP16, kind="ExternalInput")
    wq_d = nc.dram_tensor("wq", [128, KCH, 3, 128], FP16, kind="ExternalInput")
    wk_d = nc.dram_tensor("wk", [128, KCH, 3, 128], FP16, kind="ExternalInput")
    wv_d = nc.dram_tensor("wv", [128, KCH, VC], FP16, kind="ExternalInput")
    wp_d = nc.dram_tensor("wp", [128, 3, E], FP16, kind="ExternalInput")
    bq_d = nc.dram_tensor("bq", [128, 3], F32, kind="ExternalInput")
    bk_d = nc.dram_tensor("bk", [128, 3], F32, kind="ExternalInput")
    ones_d = nc.dram_tensor("ones", [128, nkc, HC], FP16, kind="ExternalInput")
    y_d = nc.dram_tensor("y", [HC, 128, S], FP16, kind="ExternalOutput")

    with tile.TileContext(nc) as tc:
        _emit(nc, tc, nkc, xt_d, xtk_d, wq_d, wk_d, wv_d, wp_d, bq_d, bk_d,
              ones_d, y_d)
    nc.compile()
    return nc


def _build_schedule(nkc):
    """slot -> list of filler units. Slots are (qbp, h, kc) flattened.

    Units: ("q", sb, m), ("k", m, kb), ("v", kc), ("at", qc8, dt, qbp),
    ("pj", Et, qb). Placement rules keep each unit >= a few slots ahead
    of its first consumer (see design notes in the module docstring).
    """
    fillers = {}

    def put(qbp, h, kc, u):
        i = (qbp * HC + h) * nkc + min(kc, nkc - 1)
        fillers.setdefault(i, []).append(u)

    # v units, head-pair granular: pair p needed by pv(h=2p) which runs
    # during head 2p+1; spread them so no single head period overloads PE
    for kc in range(nkc):
        put(0, 0, kc, ("v", kc, 0))
        put(0, 2, kc, ("v", kc, 1))
        put(0, 4, kc, ("v", kc, 2))
    # k units (prologue does m0 kb0 only); m-tile m needed by heads 2m..;
    # kb block j only feeds score chunks kc >= 4j, so later blocks are JIT
    nkb = (nkc + 3) // 4
    for j in range(1, nkb):
        put(0, 0, 2 * j - 1, ("k", 0, j))
    put(0, 1, 0, ("k", 1, 0))
    put(0, 1, 6, ("k", 1, 1))
    put(0, 2, 1, ("k", 1, 2))
    put(0, 3, 5, ("k", 2, 0))
    put(0, 3, 7, ("k", 2, 1))
    put(0, 4, 1, ("k", 2, 2))
    # q units (prologue does sb0/sb1 m0); m-tile m needed by heads 2m
    put(0, 1, 2, ("q", 0, 1))
    put(0, 1, 4, ("q", 1, 1))
    put(0, 3, 1, ("q", 0, 2))
    put(0, 3, 3, ("q", 1, 2))
    put(0, 5, 1, ("q", 2, 0))
    put(0, 5, 3, ("q", 3, 0))
    put(1, 0, 1, ("q", 2, 1))
    put(1, 0, 3, ("q", 3, 1))
    put(1, 1, 1, ("q", 2, 2))
    put(1, 1, 3, ("q", 3, 2))
    # attT transposes: (qc8, dt) one head-period after norm(2dt+1, qc8)
    for qc8 in range(8):
        put(0, 3, qc8, ("at", qc8, 0, 0))
        put(0, 5, qc8, ("at", qc8, 1, 0))
        put(1, 1, qc8, ("at", qc8, 2, 0))
        put(1, 3, qc8, ("at", qc8, 0, 1))
        put(1, 5, qc8, ("at", qc8, 1, 1))
    # proj qb0/qb1 spread through the (light) qbp1 head periods
    for Et in range(6):
        if Et < 3:
            put(1, 1, 5 + Et, ("pj", Et, 0))
        else:
            put(1, 2, 2 * (Et - 3) + 1, ("pj", Et, 0))
        if Et < 4:
            put(1, 3, 2 * Et + 1, ("pj", Et, 1))
        else:
            put(1, 4, 2 * (Et - 4) + 1, ("pj", Et, 1))
    return fillers


def _emit(nc, tc, nkc, xt_d, xtk_d, wq_d, wk_d, wv_d, wp_d, bq_d, bk_d,
          ones_d, y_d):
    kp = nkc * 128
    nkb = (kp + 511) // 512  # k-unit key blocks (512-wide, last ragged)
    ctx_pools = []

    def pool(name, bufs, space="SBUF"):
        p = tc.tile_pool(name=name, bufs=bufs, space=space)
        ctx_pools.append(p)
        return p.__enter__()

    consts = pool("consts", 1)
    store = pool("store", 1)
    pt_p = pool("pt", 2)
    sc_p = pool("sc", 2, space="PSUM")    # [128,1024] f32 = 2 banks each
    pv_p = pool("pv", 2, space="PSUM")    # [128,512] f32 = 1 bank each
    un_p = pool("un", 2, space="PSUM")    # [128,512] f32 = 1 bank each
    ys_p = pool("ys", 4)
    rs_p = pool("rs", 2)

    ident = consts.tile([128, 128], FP16)
    wq = consts.tile([128, KCH, 3, 128], FP16)
    wk = consts.tile([128, KCH, 3, 128], FP16)
    wv = consts.tile([128, KCH, VC], FP16)
    wp = consts.tile([128, 3, E], FP16)
    bq = consts.tile([128, 3], F32)
    bk = consts.tile([128, 3], F32)
    ones = consts.tile([128, nkc, HC], FP16)

    # The modeled DMA bus is near serial and only per-queue FIFO order is
    # controllable (SWDGE desc-gen has no waits, so it races the bus), so
    # ALL input loads go on the sync queue in exact priority order: the
    # critical path to the first exp (wk m0, xtk c0, wq m0, xt sb0/sb1)
    # first, then everything else by first use.


    xts = store.tile([128, KCH, S], FP16, name="xts")
    xtk = store.tile([128, KCH, kp], FP16, name="xtk")
    qT = [store.tile([128, 3, 512], FP16, name=f"qT{sb}") for sb in range(4)]
    kT = store.tile([128, 3, kp], FP16, name="kT")
    vst = store.tile([128, nkc, HC, VW], FP16, name="vst")
    att = store.tile([128, 16, VC], FP16, name="att")
    attT = store.tile([128, 3, S], FP16, name="attT")

    # x loads on the sync queue, halves first so q-unit matmuls can start
    # as soon as the first three contraction chunks land
    def load_xt(sb):
        for half in range(2):
            ks = slice(3 * half, 3 * half + 3)
            nc.sync.dma_start(
                xts[:, ks, sb * 512 : (sb + 1) * 512], xt_d.ap()[sb][:, ks, :]
            )

    def load_xtk(c0, c1):
        nc.sync.dma_start(xtk[:, :, c0:c1], xtk_d.ap()[:, :, c0:c1])

    nc.sync.dma_start(wk[:, :, 0, :], wk_d.ap()[:, :, 0, :])
    # first key block in contraction-halves so the first k-unit matmuls
    # start one transfer earlier
    nc.sync.dma_start(xtk[:, 0:3, 0:512], xtk_d.ap()[:, 0:3, 0:512])
    nc.sync.dma_start(xtk[:, 3:6, 0:512], xtk_d.ap()[:, 3:6, 0:512])
    nc.sync.dma_start(bk[:], bk_d.ap())
    nc.sync.dma_start(wq[:, :, 0, :], wq_d.ap()[:, :, 0, :])
    nc.sync.dma_start(bq[:], bq_d.ap())
    load_xt(0)
    load_xt(1)
    nc.sync.dma_start(ones[:], ones_d.ap())
    nc.sync.dma_start(wv[:], wv_d.ap())
    make_identity(nc, ident[:])
    if kp > 512:
        load_xtk(512, min(kp, 1024))
    nc.sync.dma_start(wq[:, :, 1:3, :], wq_d.ap()[:, :, 1:3, :])
    nc.sync.dma_start(wk[:, :, 1:3, :], wk_d.ap()[:, :, 1:3, :])
    if kp > 1024:
        load_xtk(1024, kp)
    load_xt(2)
    nc.sync.dma_start(wp[:], wp_d.ap())
    load_xt(3)

    # ---- units -----------------------------------------------------------
    def unit_q(sb, m):
        u = un_p.tile([128, 512], F32, tag="un", name=f"uq{sb}_{m}")
        for k in range(KCH):
            nc.tensor.matmul(
                u[:], wq[:, k, m, :], xts[:, k, sb * 512 : (sb + 1) * 512],
                start=(k == 0), stop=(k == KCH - 1),
            )
        nc.vector.tensor_scalar_add(qT[sb][:, m, :], u[:], bq[:, m : m + 1])

    def unit_k(m, kb):
        c0, c1 = kb * 512, min((kb + 1) * 512, kp)
        u = un_p.tile([128, 512], F32, tag="un", name=f"uk{m}_{kb}")
        for k in range(KCH):
            nc.tensor.matmul(
                u[:, 0 : c1 - c0], wk[:, k, m, :], xtk[:, k, c0:c1],
                start=(k == 0), stop=(k == KCH - 1),
            )
        nc.vector.tensor_scalar_add(
            kT[:, m, c0:c1], u[:, 0 : c1 - c0], bk[:, m : m + 1]
        )

    def unit_v(kc, p):
        # one head-pair's v columns: keeps the v work out of the first
        # head period (pv of head h only needs pair h//2's columns)
        u = un_p.tile([128, 512], F32, tag="un", name=f"uv{kc}_{p}")
        for k in range(KCH):
            nc.tensor.matmul(
                u[:, 0:128], xtk[:, k, kc * 128 : (kc + 1) * 128],
                wv[:, k, p * 128 : (p + 1) * 128],
                start=(k == 0), stop=(k == KCH - 1),
            )
        nc.vector.tensor_copy(
            vst[:, kc, 2 * p : 2 * p + 2, 0:D],
            u[:, 0:128].rearrange("p (h d) -> p h d", h=2),
        )
        nc.vector.tensor_copy(
            vst[:, kc, 2 * p : 2 * p + 2, D : D + 1],
            ones[:, kc : kc + 1, 2 * p : 2 * p + 2].rearrange(
                "p one h -> p h one"
            ),
        )

    def unit_attT(qc8, dt, qbp, pool=None):
        qc = qbp * 8 + qc8
        tr = (pool or un_p).tile([128, 128], FP16,
                                 tag="sc" if pool is sc_p else "un",
                                 name=f"tr{qc}_{dt}")
        nc.tensor.matmul(
            tr[:], att[:, qc, dt * 128 : (dt + 1) * 128], ident[:],
            is_transpose=True, start=True, stop=True,
        )
        nc.vector.tensor_copy(attT[:, dt, qc * 128 : (qc + 1) * 128], tr[:])

    def unit_proj(Et, qb, pool=None, evac=None):
        u = (pool or un_p).tile([128, 512], F32,
                                tag="pv" if pool is pv_p else "un",
                                name=f"up{Et}_{qb}")
        for dt in range(3):
            nc.tensor.matmul(
                u[:], wp[:, dt, Et * 128 : (Et + 1) * 128],
                attT[:, dt, qb * 512 : (qb + 1) * 512],
                start=(dt == 0), stop=(dt == 2),
            )
        ys = ys_p.tile([128, 512], FP16, tag="ys", name="ys")
        if evac is nc.scalar:
            nc.scalar.copy(ys[:], u[:])
            # keep the y-store config off the ACT SEQ (it would serialize
            # with the evacuation copies)
            nc.sync.dma_start(y_d.ap()[Et][:, qb * 512 : (qb + 1) * 512],
                              ys[:])
        else:
            nc.vector.tensor_copy(ys[:], u[:])
            eng = nc.sync if (Et + qb) % 2 == 0 else nc.scalar
            eng.dma_start(y_d.ap()[Et][:, qb * 512 : (qb + 1) * 512], ys[:])

    def pv_mms(acc, pt, h, qcs, kcs, start, stop):
        n = len(qcs)
        for ki, kc in enumerate(kcs):
            for x, qc in enumerate(qcs):
                nc.tensor.matmul(
                    acc[:, x * VW : (x + 1) * VW],
                    pt[:, kc, (qc % 8) * 128 : (qc % 8 + 1) * 128],
                    vst[:, kc, h, :],
                    start=(start and ki == 0 and x == 0),
                    stop=(stop and ki == len(kcs) - 1 and x == n - 1),
                )

    def pv_norms(acc, h, qcs):
        for x, qc in enumerate(qcs):
            rse = rs_p.tile([128, 1], F32, tag="rs", name="rse")
            with nc.allow_low_precision(reason="f32r is full width"):
                nc.vector.reciprocal(rse[:], acc[:, x * VW + D : x * VW + D + 1])
            nc.vector.tensor_scalar_mul(
                att[:, qc, h * D : (h + 1) * D],
                acc[:, x * VW : x * VW + D], rse[:],
            )

    def pv_group(pt, h, qcs):
        # one PSUM bank accumulates len(qcs) (<=2) query chunks: a single
        # start/stop accumulation group, halving pv ring turnover
        acc = pv_p.tile([128, 512], F32, tag="pv", name=f"pv{qcs[0]}_{h}")
        pv_mms(acc, pt, h, qcs, range(nkc), True, True)
        pv_norms(acc, h, qcs)

    def emit_unit(u):
        kind = u[0]
        if kind == "q":
            unit_q(u[1], u[2])
        elif kind == "k":
            unit_k(u[1], u[2])
        elif kind == "v":
            unit_v(u[1], u[2])
        elif kind == "at":
            unit_attT(u[1], u[2], u[3])
        elif kind == "pj":
            unit_proj(u[1], u[2])

    # ---- prologue units --------------------------------------------------
    unit_k(0, 0)
    unit_q(0, 0)

    fillers = _build_schedule(nkc)
    nslots = 2 * HC * nkc
    sc_pending = []  # score tiles awaiting their exp, FIFO

    def scores_mm(st, flat, j):
        qbp, rem = divmod(flat, HC * nkc)
        h, kc = divmod(rem, nkc)
        hp, r0 = h // 2, (h % 2) * 64
        sb = 2 * qbp + j
        nc.tensor.matmul(
            st[:, j * 512 : (j + 1) * 512],
            kT[r0 : r0 + 64, hp, kc * 128 : (kc + 1) * 128],
            qT[sb][r0 : r0 + 64, hp, :],
            start=True, stop=True,
        )

    def emit_scores(flat):
        st = sc_p.tile([128, 1024], F32, tag="sc", name="st")
        scores_mm(st, flat, 0)
        scores_mm(st, flat, 1)
        sc_pending.append(st)

    # front pipeline: the j0 halves of the first two score chunks depend
    # only on the sb0 query block (early on the DMA bus); q(1,0) and the
    # j1 halves follow once sb1 lands. Cross-engine deps are emission-
    # ordered, so this ordering is what lets ACT start ~2us earlier.
    if nkc >= 2:
        st0 = sc_p.tile([128, 1024], F32, tag="sc", name="st0")
        st1 = sc_p.tile([128, 1024], F32, tag="sc", name="st1")
        scores_mm(st0, 0, 0)
        scores_mm(st1, 1, 0)
    else:
        unit_q(1, 0)
        emit_scores(0)

    # ---- main loop: exp(i) -> scores(i+1) -> pv -> fillers ---------------
    pt_prev = None  # pt tile whose pv groups run during this head period
    pt_cur = None
    # last head: its pv pairs accumulate DURING its own slots ("tracking"),
    # using the idle un ring for pairs 0/1 and the pv ring (as its previous
    # user drains) for pairs 2/3 -- the tail then starts at norms directly
    track = [None] * 4
    talloc = {0: 0, 1: 0, 2: 4, 3: 5}  # pair -> first slot (catch-up there)
    for qbp in range(2):
        for h in range(HC):
            tracking = qbp == 1 and h == HC - 1 and nkc >= 6
            pt_prev = pt_cur
            pt_cur = pt_p.tile([128, nkc, 1024], FP16, tag="pt",
                               name=f"pt{qbp}_{h}")
            for kc in range(nkc):
                i = (qbp * HC + h) * nkc + kc
                if i == 0 and nkc >= 2:
                    # first two slots: exp in 512-wide halves, j0 halves
                    # first -- the sb0 query block lands well before sb1 on
                    # the serialized DMA bus, so ACT starts ~4us earlier
                    for stx, kx in ((st0, 0), (st1, 1)):
                        nc.scalar.activation(
                            pt_cur[:, kx, 0:512], stx[:, 0:512], Act.Exp,
                            scale=0.125,
                        )
                    unit_q(1, 0)
                    scores_mm(st0, 0, 1)
                    scores_mm(st1, 1, 1)
                    emit_scores(2)
                    for stx, kx in ((st0, 0), (st1, 1)):
                        nc.scalar.activation(
                            pt_cur[:, kx, 512:1024], stx[:, 512:1024],
                            Act.Exp, scale=0.125,
                        )
                    for u in fillers.get(0, ()):
                        emit_unit(u)
                    continue
                if i == 1 and nkc >= 2:
                    for u in fillers.get(1, ()):
                        emit_unit(u)
                    continue
                st = sc_pending.pop(0)
                nc.scalar.activation(pt_cur[:, kc, :], st[:], Act.Exp,
                                     scale=0.125)
                if i + 1 < nslots:
                    emit_scores(i + 1)
                if pt_prev is not None:
                    if kc < min(4, nkc - 1):
                        pvs = [2 * kc, 2 * kc + 1]
                    elif kc == nkc - 1:
                        pvs = list(range(min(8, 2 * (nkc - 1)), 8))
                    else:
                        pvs = []
                    ph = (h - 1) % HC
                    pqbp = qbp if h > 0 else qbp - 1
                    for x in range(0, len(pvs), 2):
                        pv_group(pt_prev, ph,
                                 [pqbp * 8 + q for q in pvs[x : x + 2]])
                if tracking:
                    for j in range(4):
                        k0 = talloc[j]
                        if kc < k0:
                            continue
                        qcs = [8 + 2 * j, 9 + 2 * j]
                        if kc == k0:
                            pl, tg = (un_p, "un") if j < 2 else (pv_p, "pv")
                            track[j] = pl.tile([128, 512], F32, tag=tg,
                                               name=f"tk{j}")
                            pv_mms(track[j], pt_cur, h, qcs,
                                   range(0, k0 + 1), True, kc == nkc - 1)
                        else:
                            pv_mms(track[j], pt_cur, h, qcs, [kc], False,
                                   kc == nkc - 1)
                for u in fillers.get(i, ()):
                    if tracking and u[0] == "at":
                        unit_attT(u[1], u[2], u[3], pool=sc_p)
                    else:
                        emit_unit(u)

    # ---- tail: last head's pv + attT dt2 + proj qb2/qb3 ------------------
    # interleave so PE never sits on the pv->norm->attT DVE chains: each
    # attT lags its pv by one group, proj units weave between pv groups as
    # soon as their four attT columns are present.
    if nkc >= 6:
        # tracked accumulators are complete right after the last exp: the
        # tail is just norms -> attT dt2 -> proj qb2/qb3, with the proj
        # units rotating over both freed PSUM rings (4 banks)
        pv_norms(track[0], HC - 1, [8, 9])
        pv_norms(track[1], HC - 1, [10, 11])
        for qc8 in range(4):
            unit_attT(qc8, 2, 1, pool=sc_p)
        pv_norms(track[2], HC - 1, [12, 13])
        pv_norms(track[3], HC - 1, [14, 15])
        ys2 = store.tile([128, HC, 512], FP16, name="ys2")

        def proj_qb2(Et, pl):
            u = pl.tile([128, 512], F32, tag="pv" if pl is pv_p else "un",
                        name=f"up{Et}_2")
            for dt in range(3):
                nc.tensor.matmul(
                    u[:], wp[:, dt, Et * 128 : (Et + 1) * 128],
                    attT[:, dt, 2 * 512 : 3 * 512],
                    start=(dt == 0), stop=(dt == 2),
                )
            if Et % 2 == 0:
                nc.scalar.copy(ys2[:, Et, :], u[:])
            else:
                nc.vector.tensor_copy(ys2[:, Et, :], u[:])
            if Et == 2:
                nc.sync.dma_start(
                    y_d.ap()[0:3, :, 2 * 512 : 3 * 512].rearrange(
                        "e p s -> p e s"
                    ),
                    ys2[:, 0:3, :],
                )
            elif Et == 5:
                nc.sync.dma_start(
                    y_d.ap()[3:6, :, 2 * 512 : 3 * 512].rearrange(
                        "e p s -> p e s"
                    ),
                    ys2[:, 3:6, :],
                )

        proj_qb2(0, un_p)
        proj_qb2(1, un_p)
        for qc8 in range(4, 8):
            unit_attT(qc8, 2, 1, pool=sc_p)
        for Et, pl in ((2, pv_p), (3, pv_p), (4, un_p), (5, un_p)):
            proj_qb2(Et, pl)
        # final query block: evacuate into one staging tile and store in
        # two 3-tile DMAs -- per-store HWDGE configs would serialize the
        # end-of-kernel chain
        ys6 = store.tile([128, HC, 512], FP16, name="ys6")
        for Et, pl in ((0, pv_p), (1, pv_p), (2, un_p), (3, un_p),
                       (4, pv_p), (5, pv_p)):
            u = pl.tile([128, 512], F32, tag="pv" if pl is pv_p else "un",
                        name=f"up{Et}_3")
            for dt in range(3):
                nc.tensor.matmul(
                    u[:], wp[:, dt, Et * 128 : (Et + 1) * 128],
                    attT[:, dt, 3 * 512 : 4 * 512],
                    start=(dt == 0), stop=(dt == 2),
                )
            if Et % 2 == 0:
                nc.scalar.copy(ys6[:, Et, :], u[:])
            else:
                nc.vector.tensor_copy(ys6[:, Et, :], u[:])
            if Et == 2:
                nc.sync.dma_start(
                    y_d.ap()[0:3, :, 3 * 512 : 4 * 512].rearrange(
                        "e p s -> p e s"
                    ),
                    ys6[:, 0:3, :],
                )
        nc.sync.dma_start(
            y_d.ap()[3:6, :, 3 * 512 : 4 * 512].rearrange("e p s -> p e s"),
            ys6[:, 3:6, :],
        )
    else:
        for pp in range(4):
            pv_group(pt_cur, HC - 1, [8 + 2 * pp, 9 + 2 * pp])
            if pp >= 1:
                unit_attT(2 * pp - 2, 2, 1)
                unit_attT(2 * pp - 1, 2, 1)
        unit_attT(6, 2, 1)
        unit_attT(7, 2, 1)
        for Et in range(6):
            unit_proj(Et, 2)
        for Et in range(6):
            unit_proj(Et, 3)

    for p in reversed(ctx_pools):
        p.__exit__(None, None, None)


def make_core_inputs(x, mask, Wqkv, bqkv, Wproj, kp):
    """Slice full inputs into 8 per-core input maps (host-side layouts)."""
    x = np.asarray(x, np.float32)
    mask = np.asarray(mask)
    Wqkv = np.asarray(Wqkv, np.float32)
    bqkv = np.asarray(bqkv, np.float32)
    Wproj = np.asarray(Wproj, np.float32)
    nkc = kp // 128
    f16 = np.float16
    maps = []
    for c in range(8):
        b, hg = c // 2, c % 2
        h0 = hg * HC
        keep = np.nonzero(mask[b, 0, 0, :] != 0)[0]
        kept = len(keep)
        xt = x[b].T.reshape(KCH, 128, S).transpose(1, 0, 2)  # [p, kch, s]
        xt4 = np.ascontiguousarray(
            xt.reshape(128, KCH, 4, 512).transpose(2, 0, 1, 3).astype(f16)
        )
        xk = np.zeros((kp, E), np.float32)
        xk[:kept] = x[b, keep, :]
        xtk = np.ascontiguousarray(
            xk.T.reshape(KCH, 128, kp).transpose(1, 0, 2).astype(f16)
        )
        wq = Wqkv[:, h0 * D : (h0 + HC) * D]
        wq = np.ascontiguousarray(
            wq.reshape(KCH, 128, 3, 128).transpose(1, 0, 2, 3).astype(f16)
        )
        wkk = Wqkv[:, E + h0 * D : E + (h0 + HC) * D]
        wkk = np.ascontiguousarray(
            wkk.reshape(KCH, 128, 3, 128).transpose(1, 0, 2, 3).astype(f16)
        )
        wvv = Wqkv[:, 2 * E + h0 * D : 2 * E + (h0 + HC) * D]
        wvv = np.ascontiguousarray(
            wvv.reshape(KCH, 128, VC).transpose(1, 0, 2).astype(f16)
        )
        wpp = Wproj[hg * VC : (hg + 1) * VC, :]
        wpp = np.ascontiguousarray(
            wpp.reshape(3, 128, E).transpose(1, 0, 2).astype(f16)
        )
        bqq = np.ascontiguousarray(
            bqkv[h0 * D : (h0 + HC) * D].reshape(3, 128).T.astype(np.float32)
        )
        bkk = np.ascontiguousarray(
            bqkv[E + h0 * D : E + (h0 + HC) * D]
            .reshape(3, 128).T.astype(np.float32)
        )
        keepmask = (np.arange(kp) < kept).astype(f16).reshape(nkc, 128).T
        onesr = np.ascontiguousarray(
            np.repeat(keepmask[:, :, None], HC, axis=2).astype(f16)
        )
        maps.append(
            {
                "xt": xt4, "xtk": xtk, "wq": wq, "wk": wkk, "wv": wvv,
                "wp": wpp, "bq": bqq, "bk": bkk, "ones": onesr,
            }
        )
    return maps


def run(x, mask, Wqkv, bqkv, Wproj, bproj, trace=False, trace_cores=None):
    mask = np.asarray(mask)
    Wproj_np = np.asarray(Wproj, np.float32)
    bproj_np = np.asarray(bproj, np.float32)
    bqkv_np = np.asarray(bqkv, np.float32)
    kept = (mask[:, 0, 0, :] != 0).sum(axis=1)
    kp = max(128, int(-(-kept.max() // 128)) * 128)
    in_maps = make_core_inputs(x, mask, Wqkv, bqkv_np, Wproj_np, kp)

    nc = build_program(kp)
    try:
        res = run_bass_kernel_spmd(
            nc, in_maps, core_ids=list(range(8)), trace=trace,
            trace_cores=trace_cores,
        )
    except Exception:
        # transient device wedge -- one retry is usually enough
        res = run_bass_kernel_spmd(
            nc, in_maps, core_ids=list(range(8)), trace=trace,
            trace_cores=trace_cores,
        )

    # host-folded bias: v-bias passes through softmax (weights sum to 1)
    bv = bqkv_np[2 * E : 3 * E]
    bias_row = bv @ Wproj_np + bproj_np
    y = np.empty((B, S, E), np.float32)
    for b in range(B):
        p0 = res.results[2 * b]["y"].reshape(E, S).astype(np.float32)
        p1 = res.results[2 * b + 1]["y"].reshape(E, S).astype(np.float32)
        y[b] = p0.T + p1.T + bias_row
    return y, res


def kernel(x, mask, Wqkv, bqkv, Wproj, bproj):
    y, _ = run(x, mask, Wqkv, bqkv, Wproj, bproj, trace=False)
    return y


# revision 65
# speedup vs baseline: 1.0149x; 1.0149x over previous
"""Trainium2 Bass kernel for CodeAttention (B=4, S=2048, E=768, H=12).

Sharding: 8 cores = 4 batches x 2 head-groups (6 heads each). Each core
computes a partial projection output for its batch; the host sums the two
partials per batch and adds the (host-folded) bias row.

Design (fp16 datapath; ~149us/core cost-model estimate vs the 270us
fused baseline; max rel err ~6.7e-4):
- Key compaction: the padding mask is known on the host, so masked keys
  (~50%) are gathered OUT of the K/V stream entirely (exact math: they
  contribute to neither the numerator nor the softmax denominator). Kept
  keys are padded to KP (multiple of 128) with zero columns whose ones-
  column entry is 0, which keeps them exactly inert.
- pv orientation flip: out[q,65] = sum_k pt[k,q]*vst[k,65] makes the
  moving operand the 65-wide V tile, cutting pv PE rows ~2x vs moving
  the 512-wide query block. The 65th column accumulates the softmax
  denominator, so normalization is a per-partition reciprocal+scale on
  DVE (no gpsimd broadcast). Two query-chunk accumulators share each
  PSUM bank under a single start/stop group.
- x arrives pre-transposed from the host (xt, xtk), so there are no
  on-chip x transposes; att is re-transposed on PE (48 small transposes)
  for the output projection, and y leaves as yT (host re-transposes).
- Main rhythm: per (query-half, head) 9 key-chunk slots: exp(kc) on ACT,
  then scores(kc+1) (one slot of lookahead keeps ACT fed -- cross-engine
  deps are emission-ordered counters), then pv groups of the previous
  head, then statically scheduled filler units (q/k/v/proj/attT) spread
  so no head period overloads PE (v is head-pair granular).
- The modeled DMA bus is near serial: all input loads ride one queue in
  exact priority order (first-exp critical path leads); the first two
  exps are split into 512-wide halves so ACT starts before xt-sb1 lands.
- Last head: its pv pairs accumulate DURING its own slots (un ring +
  drained pv ring), so the tail is norms -> attT dt2 -> output
  projections, with the final stores batched into 3-tile DMAs.
"""

import sys

if "/opt/trn_rl_repo" not in sys.path:
    sys.path.insert(0, "/opt/trn_rl_repo")

import numpy as np

import concourse.bass as bass  # noqa: F401
import concourse.mybir as mybir
import concourse.tile as tile
from concourse import bacc
from concourse.alu_op_type import AluOpType
from concourse.bass_utils import run_bass_kernel_spmd
from concourse.masks import make_identity

F32 = mybir.dt.float32
F32R = mybir.dt.float32r
FP16 = mybir.dt.float16
Act = mybir.ActivationFunctionType

B, S, E, H, D = 4, 2048, 768, 12, 64
HC = 6                    # heads per core
KCH = E // 128            # contraction chunks over E = 6
VC = HC * D               # v columns per core = 384
VW = D + 1                # v width incl. ones column = 65
DEFAULT_KP = 1152         # padded kept-key count for the fixed-seed mask


def build_program(kp=DEFAULT_KP):
    nkc = kp // 128
    nc = bacc.Bacc("TRN2", target_bir_lowering=False, debug=False, num_devices=8)

    xt_d = nc.dram_tensor("xt", [4, 128, KCH, 512], FP16, kind="ExternalInput")
    xtk_d = nc.dram_tensor("xtk", [128, KCH, kp], FP16, kind="ExternalInput")
    wq_d = nc.dram_tensor("wq", [128, KCH, 3, 128], FP16, kind="ExternalInput")
    wk_d = nc.dram_tensor("wk", [128, KCH, 3, 128], FP16, kind="ExternalInput")
    wv_d = nc.dram_tensor("wv", [128, KCH, VC], FP16, kind="ExternalInput")
    wp_d = nc.dram_tensor("wp", [128, 3, E], FP16, kind="ExternalInput")
    bq_d = nc.dram_tensor("bq", [128, 3], F32, kind="ExternalInput")
    bk_d = nc.dram_tensor("bk", [128, 3], F32, kind="ExternalInput")
    ones_d = nc.dram_tensor("ones", [128, nkc, HC], FP16, kind="ExternalInput")
    y_d = nc.dram_tensor("y", [HC, 128, S], FP16, kind="ExternalOutput")

    with tile.TileContext(nc) as tc:
        _emit(nc, tc, nkc, xt_d, xtk_d, wq_d, wk_d, wv_d, wp_d, bq_d, bk_d,
              ones_d, y_d)
    nc.compile()
    return nc


def _build_schedule(nkc):
    """slot -> list of filler units. Slots are (qbp, h, kc) flattened.

    Units: ("q", sb, m), ("k", m, kb), ("v", kc), ("at", qc8, dt, qbp),
    ("pj", Et, qb). Placement rules keep each unit >= a few slots ahead
    of its first consumer (see design notes in the module docstring).
    """
    fillers = {}

    def put(qbp, h, kc, u):
        i = (qbp * HC + h) * nkc + min(kc, nkc - 1)
        fillers.setdefault(i, []).append(u)

    # v units, head-pair granular: pair p needed by pv(h=2p) which runs
    # during head 2p+1; spread them so no single head period overloads PE
    for kc in range(nkc):
        put(0, 0, kc, ("v", kc, 0))
        put(0, 2, kc, ("v", kc, 1))
        put(0, 4, kc, ("v", kc, 2))
    # k units (prologue does m0 kb0 only); m-tile m needed by heads 2m..;
    # kb block j only feeds score chunks kc >= 4j, so later blocks are JIT
    nkb = (nkc + 3) // 4
    for j in range(1, nkb):
        put(0, 0, 2 * j - 1, ("k", 0, j))
    put(0, 1, 0, ("k", 1, 0))
    put(0, 1, 6, ("k", 1, 1))
    put(0, 2, 1, ("k", 1, 2))
    put(0, 3, 5, ("k", 2, 0))
    put(0, 3, 7, ("k", 2, 1))
    put(0, 4, 1, ("k", 2, 2))
    # q units (prologue does sb0/sb1 m0); m-tile m needed by heads 2m
    put(0, 1, 2, ("q", 0, 1))
    put(0, 1, 4, ("q", 1, 1))
    put(0, 3, 1, ("q", 0, 2))
    put(0, 3, 3, ("q", 1, 2))
    put(0, 5, 1, ("q", 2, 0))
    put(0, 5, 3, ("q", 3, 0))
    put(1, 0, 1, ("q", 2, 1))
    put(1, 0, 3, ("q", 3, 1))
    put(1, 1, 1, ("q", 2, 2))
    put(1, 1, 3, ("q", 3, 2))
    # attT transposes: (qc8, dt) one head-period after norm(2dt+1, qc8).
    # dt1 of the second query half moves INTO (1,4): its pv-pair norms
    # land at slot qc8//2 there, and (1,5) must keep the un ring free for
    # the tracking accumulators (and its sc ring free for scores).
    for qc8 in range(8):
        put(0, 3, qc8, ("at", qc8, 0, 0))
        put(0, 5, qc8, ("at", qc8, 1, 0))
        put(1, 1, qc8, ("at", qc8, 2, 0))
        put(1, 3, qc8, ("at", qc8, 0, 1))
        put(1, 4, qc8 // 2 + 1, ("at", qc8, 1, 1))
    # proj qb0/qb1 spread through the (light) qbp1 head periods
    for Et in range(6):
        if Et < 3:
            put(1, 1, 5 + Et, ("pj", Et, 0))
        else:
            put(1, 2, 2 * (Et - 3) + 1, ("pj", Et, 0))
        if Et < 4:
            put(1, 3, 2 * Et + 1, ("pj", Et, 1))
        else:
            put(1, 4, 2 * (Et - 4) + 1, ("pj", Et, 1))
    return fillers


def _emit(nc, tc, nkc, xt_d, xtk_d, wq_d, wk_d, wv_d, wp_d, bq_d, bk_d,
          ones_d, y_d):
    kp = nkc * 128
    nkb = (kp + 511) // 512  # k-unit key blocks (512-wide, last ragged)
    ctx_pools = []

    def pool(name, bufs, space="SBUF"):
        p = tc.tile_pool(name=name, bufs=bufs, space=space)
        ctx_pools.append(p)
        return p.__enter__()

    consts = pool("consts", 1)
    store = pool("store", 1)
    pt_p = pool("pt", 2)
    sc_p = pool("sc", 2, space="PSUM")    # [128,1024] f32 = 2 banks each
    pv_p = pool("pv", 2, space="PSUM")    # [128,512] f32 = 1 bank each
    un_p = pool("un", 2, space="PSUM")    # [128,512] f32 = 1 bank each
    ys_p = pool("ys", 4)
    rs_p = pool("rs", 2)

    ident = consts.tile([128, 128], FP16)
    wq = consts.tile([128, KCH, 3, 128], FP16)
    wk = consts.tile([128, KCH, 3, 128], FP16)
    wv = consts.tile([128, KCH, VC], FP16)
    wp = consts.tile([128, 3, E], FP16)
    bq = consts.tile([128, 3], F32)
    bk = consts.tile([128, 3], F32)
    ones = consts.tile([128, nkc, HC], FP16)

    # The modeled DMA bus is near serial and only per-queue FIFO order is
    # controllable (SWDGE desc-gen has no waits, so it races the bus), so
    # ALL input loads go on the sync queue in exact priority order: the
    # critical path to the first exp (wk m0, xtk c0, wq m0, xt sb0/sb1)
    # first, then everything else by first use.


    xts = store.tile([128, KCH, S], FP16, name="xts")
    xtk = store.tile([128, KCH, kp], FP16, name="xtk")
    qT = [store.tile([128, 3, 512], FP16, name=f"qT{sb}") for sb in range(4)]
    kT = store.tile([128, 3, kp], FP16, name="kT")
    vst = store.tile([128, nkc, HC, VW], FP16, name="vst")
    att = store.tile([128, 16, VC], FP16, name="att")
    attT = store.tile([128, 3, S], FP16, name="attT")

    # x loads on the sync queue, halves first so q-unit matmuls can start
    # as soon as the first three contraction chunks land
    def load_xt(sb):
        for half in range(2):
            ks = slice(3 * half, 3 * half + 3)
            nc.sync.dma_start(
                xts[:, ks, sb * 512 : (sb + 1) * 512], xt_d.ap()[sb][:, ks, :]
            )

    def load_xtk(c0, c1):
        nc.sync.dma_start(xtk[:, :, c0:c1], xtk_d.ap()[:, :, c0:c1])

    nc.sync.dma_start(wk[:, :, 0, :], wk_d.ap()[:, :, 0, :])
    # first key block in contraction-halves so the first k-unit matmuls
    # start one transfer earlier
    nc.sync.dma_start(xtk[:, 0:3, 0:512], xtk_d.ap()[:, 0:3, 0:512])
    nc.sync.dma_start(xtk[:, 3:6, 0:512], xtk_d.ap()[:, 3:6, 0:512])
    nc.sync.dma_start(bk[:], bk_d.ap())
    nc.sync.dma_start(wq[:, :, 0, :], wq_d.ap()[:, :, 0, :])
    nc.sync.dma_start(bq[:], bq_d.ap())
    load_xt(0)
    load_xt(1)
    nc.sync.dma_start(ones[:], ones_d.ap())
    nc.sync.dma_start(wv[:], wv_d.ap())
    make_identity(nc, ident[:])
    if kp > 512:
        load_xtk(512, min(kp, 1024))
    nc.sync.dma_start(wq[:, :, 1:3, :], wq_d.ap()[:, :, 1:3, :])
    nc.sync.dma_start(wk[:, :, 1:3, :], wk_d.ap()[:, :, 1:3, :])
    if kp > 1024:
        load_xtk(1024, kp)
    load_xt(2)
    nc.sync.dma_start(wp[:], wp_d.ap())
    load_xt(3)

    # ---- units -----------------------------------------------------------
    def unit_q(sb, m):
        u = un_p.tile([128, 512], F32, tag="un", name=f"uq{sb}_{m}")
        for k in range(KCH):
            nc.tensor.matmul(
                u[:], wq[:, k, m, :], xts[:, k, sb * 512 : (sb + 1) * 512],
                start=(k == 0), stop=(k == KCH - 1),
            )
        nc.vector.tensor_scalar_add(qT[sb][:, m, :], u[:], bq[:, m : m + 1])

    def unit_k(m, kb):
        c0, c1 = kb * 512, min((kb + 1) * 512, kp)
        u = un_p.tile([128, 512], F32, tag="un", name=f"uk{m}_{kb}")
        for k in range(KCH):
            nc.tensor.matmul(
                u[:, 0 : c1 - c0], wk[:, k, m, :], xtk[:, k, c0:c1],
                start=(k == 0), stop=(k == KCH - 1),
            )
        nc.vector.tensor_scalar_add(
            kT[:, m, c0:c1], u[:, 0 : c1 - c0], bk[:, m : m + 1]
        )

    def unit_v(kc, p):
        # one head-pair's v columns: keeps the v work out of the first
        # head period (pv of head h only needs pair h//2's columns)
        u = un_p.tile([128, 512], F32, tag="un", name=f"uv{kc}_{p}")
        for k in range(KCH):
            nc.tensor.matmul(
                u[:, 0:128], xtk[:, k, kc * 128 : (kc + 1) * 128],
                wv[:, k, p * 128 : (p + 1) * 128],
                start=(k == 0), stop=(k == KCH - 1),
            )
        nc.vector.tensor_copy(
            vst[:, kc, 2 * p : 2 * p + 2, 0:D],
            u[:, 0:128].rearrange("p (h d) -> p h d", h=2),
        )
        nc.vector.tensor_copy(
            vst[:, kc, 2 * p : 2 * p + 2, D : D + 1],
            ones[:, kc : kc + 1, 2 * p : 2 * p + 2].rearrange(
                "p one h -> p h one"
            ),
        )

    def unit_attT(qc8, dt, qbp, pool=None):
        qc = qbp * 8 + qc8
        tr = (pool or un_p).tile([128, 128], FP16,
                                 tag="sc" if pool is sc_p else "un",
                                 name=f"tr{qc}_{dt}")
        nc.tensor.matmul(
            tr[:], att[:, qc, dt * 128 : (dt + 1) * 128], ident[:],
            is_transpose=True, start=True, stop=True,
        )
        nc.vector.tensor_copy(attT[:, dt, qc * 128 : (qc + 1) * 128], tr[:])

    def unit_proj(Et, qb, pool=None, evac=None):
        u = (pool or un_p).tile([128, 512], F32,
                                tag="pv" if pool is pv_p else "un",
                                name=f"up{Et}_{qb}")
        for dt in range(3):
            nc.tensor.matmul(
                u[:], wp[:, dt, Et * 128 : (Et + 1) * 128],
                attT[:, dt, qb * 512 : (qb + 1) * 512],
                start=(dt == 0), stop=(dt == 2),
            )
        ys = ys_p.tile([128, 512], FP16, tag="ys", name="ys")
        if evac is nc.scalar:
            nc.scalar.copy(ys[:], u[:])
            # keep the y-store config off the ACT SEQ (it would serialize
            # with the evacuation copies)
            nc.sync.dma_start(y_d.ap()[Et][:, qb * 512 : (qb + 1) * 512],
                              ys[:])
        else:
            nc.vector.tensor_copy(ys[:], u[:])
            eng = nc.sync if (Et + qb) % 2 == 0 else nc.scalar
            eng.dma_start(y_d.ap()[Et][:, qb * 512 : (qb + 1) * 512], ys[:])

    def pv_mms(acc, pt, h, qcs, kcs, start, stop):
        n = len(qcs)
        for ki, kc in enumerate(kcs):
            for x, qc in enumerate(qcs):
                nc.tensor.matmul(
                    acc[:, x * VW : (x + 1) * VW],
                    pt[:, kc, (qc % 8) * 128 : (qc % 8 + 1) * 128],
                    vst[:, kc, h, :],
                    start=(start and ki == 0 and x == 0),
                    stop=(stop and ki == len(kcs) - 1 and x == n - 1),
                )

    def pv_norms(acc, h, qcs):
        for x, qc in enumerate(qcs):
            rse = rs_p.tile([128, 1], F32, tag="rs", name="rse")
            with nc.allow_low_precision(reason="f32r is full width"):
                nc.vector.reciprocal(rse[:], acc[:, x * VW + D : x * VW + D + 1])
            nc.vector.tensor_scalar_mul(
                att[:, qc, h * D : (h + 1) * D],
                acc[:, x * VW : x * VW + D], rse[:],
            )

    def pv_group(pt, h, qcs):
        # one PSUM bank accumulates len(qcs) (<=2) query chunks: a single
        # start/stop accumulation group, halving pv ring turnover
        acc = pv_p.tile([128, 512], F32, tag="pv", name=f"pv{qcs[0]}_{h}")
        pv_mms(acc, pt, h, qcs, range(nkc), True, True)
        pv_norms(acc, h, qcs)

    def emit_unit(u):
        kind = u[0]
        if kind == "q":
            unit_q(u[1], u[2])
        elif kind == "k":
            unit_k(u[1], u[2])
        elif kind == "v":
            unit_v(u[1], u[2])
        elif kind == "at":
            unit_attT(u[1], u[2], u[3])
        elif kind == "pj":
            unit_proj(u[1], u[2])

    # ---- prologue units --------------------------------------------------
    unit_k(0, 0)
    unit_q(0, 0)

    fillers = _build_schedule(nkc)
    nslots = 2 * HC * nkc
    sc_pending = []  # score tiles awaiting their exp, FIFO

    def scores_mm(st, flat, j):
        qbp, rem = divmod(flat, HC * nkc)
        h, kc = divmod(rem, nkc)
        hp, r0 = h // 2, (h % 2) * 64
        sb = 2 * qbp + j
        nc.tensor.matmul(
            st[:, j * 512 : (j + 1) * 512],
            kT[r0 : r0 + 64, hp, kc * 128 : (kc + 1) * 128],
            qT[sb][r0 : r0 + 64, hp, :],
            start=True, stop=True,
        )

    def emit_scores(flat):
        st = sc_p.tile([128, 1024], F32, tag="sc", name="st")
        scores_mm(st, flat, 0)
        scores_mm(st, flat, 1)
        sc_pending.append(st)

    # front pipeline: the j0 halves of the first two score chunks depend
    # only on the sb0 query block (early on the DMA bus); q(1,0) and the
    # j1 halves follow once sb1 lands. Cross-engine deps are emission-
    # ordered, so this ordering is what lets ACT start ~2us earlier.
    if nkc >= 2:
        st0 = sc_p.tile([128, 1024], F32, tag="sc", name="st0")
        st1 = sc_p.tile([128, 1024], F32, tag="sc", name="st1")
        scores_mm(st0, 0, 0)
        scores_mm(st1, 1, 0)
    else:
        unit_q(1, 0)
        emit_scores(0)

    # ---- main loop: exp(i) -> scores(i+1) -> pv -> fillers ---------------
    pt_prev = None  # pt tile whose pv groups run during this head period
    pt_cur = None
    # last head: its pv pairs accumulate DURING its own slots ("tracking"),
    # using the idle un ring for pairs 0/1 and the pv ring (as its previous
    # user drains) for pairs 2/3 -- the tail then starts at norms directly
    track = [None] * 4
    talloc = {0: 0, 1: 0, 2: 4, 3: 5}  # pair -> first slot (catch-up there)
    for qbp in range(2):
        for h in range(HC):
            tracking = qbp == 1 and h == HC - 1 and nkc >= 6
            pt_prev = pt_cur
            pt_cur = pt_p.tile([128, nkc, 1024], FP16, tag="pt",
                               name=f"pt{qbp}_{h}")
            for kc in range(nkc):
                i = (qbp * HC + h) * nkc + kc
                if i == 0 and nkc >= 2:
                    # first two slots: exp in 512-wide halves, j0 halves
                    # first -- the sb0 query block lands well before sb1 on
                    # the serialized DMA bus, so ACT starts ~4us earlier
                    for stx, kx in ((st0, 0), (st1, 1)):
                        nc.scalar.activation(
                            pt_cur[:, kx, 0:512], stx[:, 0:512], Act.Exp,
                            scale=0.125,
                        )
                    unit_q(1, 0)
                    scores_mm(st0, 0, 1)
                    scores_mm(st1, 1, 1)
                    emit_scores(2)
                    for stx, kx in ((st0, 0), (st1, 1)):
                        nc.scalar.activation(
                            pt_cur[:, kx, 512:1024], stx[:, 512:1024],
                            Act.Exp, scale=0.125,
                        )
                    for u in fillers.get(0, ()):
                        emit_unit(u)
                    continue
                if i == 1 and nkc >= 2:
                    for u in fillers.get(1, ()):
                        emit_unit(u)
                    continue
                st = sc_pending.pop(0)
                nc.scalar.activation(pt_cur[:, kc, :], st[:], Act.Exp,
                                     scale=0.125)
                if i + 1 < nslots:
                    emit_scores(i + 1)
                if pt_prev is not None:
                    if kc < min(4, nkc - 1):
                        pvs = [2 * kc, 2 * kc + 1]
                    elif kc == nkc - 1:
                        pvs = list(range(min(8, 2 * (nkc - 1)), 8))
                    else:
                        pvs = []
                    ph = (h - 1) % HC
                    pqbp = qbp if h > 0 else qbp - 1
                    for x in range(0, len(pvs), 2):
                        pv_group(pt_prev, ph,
                                 [pqbp * 8 + q for q in pvs[x : x + 2]])
                if tracking:
                    for j in range(4):
                        k0 = talloc[j]
                        if kc < k0:
                            continue
                        qcs = [8 + 2 * j, 9 + 2 * j]
                        if kc == k0:
                            pl, tg = (un_p, "un") if j < 2 else (pv_p, "pv")
                            track[j] = pl.tile([128, 512], F32, tag=tg,
                                               name=f"tk{j}")
                            pv_mms(track[j], pt_cur, h, qcs,
                                   range(0, k0 + 1), True, kc == nkc - 1)
                        else:
                            pv_mms(track[j], pt_cur, h, qcs, [kc], False,
                                   kc == nkc - 1)
                for u in fillers.get(i, ()):
                    if tracking and u[0] == "at":
                        unit_attT(u[1], u[2], u[3], pool=sc_p)
                    else:
                        emit_unit(u)

    # ---- tail: last head's pv + attT dt2 + proj qb2/qb3 ------------------
    # interleave so PE never sits on the pv->norm->attT DVE chains: each
    # attT lags its pv by one group, proj units weave between pv groups as
    # soon as their four attT columns are present.
    if nkc >= 6:
        # tracked accumulators are complete right after the last exp: the
        # tail is just norms -> attT dt2 -> proj qb2/qb3, with the proj
        # units rotating over both freed PSUM rings (4 banks)
        pv_norms(track[0], HC - 1, [8, 9])
        pv_norms(track[1], HC - 1, [10, 11])
        for qc8 in range(4):
            unit_attT(qc8, 2, 1, pool=sc_p)
        pv_norms(track[2], HC - 1, [12, 13])
        pv_norms(track[3], HC - 1, [14, 15])
        ys2 = store.tile([128, HC, 512], FP16, name="ys2")

        def proj_qb2(Et, pl):
            u = pl.tile([128, 512], F32, tag="pv" if pl is pv_p else "un",
                        name=f"up{Et}_2")
            for dt in range(3):
                nc.tensor.matmul(
                    u[:], wp[:, dt, Et * 128 : (Et + 1) * 128],
                    attT[:, dt, 2 * 512 : 3 * 512],
                    start=(dt == 0), stop=(dt == 2),
                )
            if Et % 2 == 0:
                nc.scalar.copy(ys2[:, Et, :], u[:])
            else:
                nc.vector.tensor_copy(ys2[:, Et, :], u[:])
            if Et == 2:
                nc.sync.dma_start(
                    y_d.ap()[0:3, :, 2 * 512 : 3 * 512].rearrange(
                        "e p s -> p e s"
                    ),
                    ys2[:, 0:3, :],
                )
            elif Et == 5:
                nc.sync.dma_start(
                    y_d.ap()[3:6, :, 2 * 512 : 3 * 512].rearrange(
                        "e p s -> p e s"
                    ),
                    ys2[:, 3:6, :],
                )

        proj_qb2(0, un_p)
        proj_qb2(1, un_p)
        for qc8 in range(4, 8):
            unit_attT(qc8, 2, 1, pool=sc_p)
        for Et, pl in ((2, pv_p), (3, pv_p), (4, un_p), (5, un_p)):
            proj_qb2(Et, pl)
        # final query block: evacuate into one staging tile and store in
        # two 3-tile DMAs -- per-store HWDGE configs would serialize the
        # end-of-kernel chain
        ys6 = store.tile([128, HC, 512], FP16, name="ys6")
        for Et, pl in ((0, pv_p), (1, pv_p), (2, un_p), (3, un_p),
                       (4, pv_p), (5, pv_p)):
            u = pl.tile([128, 512], F32, tag="pv" if pl is pv_p else "un",
                        name=f"up{Et}_3")
            for dt in range(3):
                nc.tensor.matmul(
                    u[:], wp[:, dt, Et * 128 : (Et + 1) * 128],
                    attT[:, dt, 3 * 512 : 4 * 512],
                    start=(dt == 0), stop=(dt == 2),
                )
            if Et % 2 == 0:
                nc.scalar.copy(ys6[:, Et, :], u[:])
            else:
                nc.vector.tensor_copy(ys6[:, Et, :], u[:])
            if Et == 2:
                nc.sync.dma_start(
                    y_d.ap()[0:3, :, 3 * 512 : 4 * 512].rearrange(
                        "e p s -> p e s"
                    ),
                    ys6[:, 0:3, :],
                )
            elif Et == 4:
                nc.sync.dma_start(
                    y_d.ap()[3:5, :, 3 * 512 : 4 * 512].rearrange(
                        "e p s -> p e s"
                    ),
                    ys6[:, 3:5, :],
                )
        # the very last store is a single tile so the end-of-kernel chain
        # rides the shortest possible transfer
        nc.sync.dma_start(
            y_d.ap()[5:6, :, 3 * 512 : 4 * 512].rearrange("e p s -> p e s"),
            ys6[:, 5:6, :],
        )
    else:
        for pp in range(4):
            pv_group(pt_cur, HC - 1, [8 + 2 * pp, 9 + 2 * pp])
            if pp >= 1:
                unit_attT(2 * pp - 2, 2, 1)
                unit_attT(2 * pp - 1, 2, 1)
        unit_attT(6, 2, 1)
        unit_attT(7, 2, 1)
        for Et in range(6):
            unit_proj(Et, 2)
        for Et in range(6):
            unit_proj(Et, 3)

    for p in reversed(ctx_pools):
        p.__exit__(None, None, None)


def make_core_inputs(x, mask, Wqkv, bqkv, Wproj, kp):
    """Slice full inputs into 8 per-core input maps (host-side layouts)."""
    x = np.asarray(x, np.float32)
    mask = np.asarray(mask)
    Wqkv = np.asarray(Wqkv, np.float32)
    bqkv = np.asarray(bqkv, np.float32)
    Wproj = np.asarray(Wproj, np.float32)
    nkc = kp // 128
    f16 = np.float16
    maps = []
    for c in range(8):
        b, hg = c // 2, c % 2
        h0 = hg * HC
        keep = np.nonzero(mask[b, 0, 0, :] != 0)[0]
        kept = len(keep)
        xt = x[b].T.reshape(KCH, 128, S).transpose(1, 0, 2)  # [p, kch, s]
        xt4 = np.ascontiguousarray(
            xt.reshape(128, KCH, 4, 512).transpose(2, 0, 1, 3).astype(f16)
        )
        xk = np.zeros((kp, E), np.float32)
        xk[:kept] = x[b, keep, :]
        xtk = np.ascontiguousarray(
            xk.T.reshape(KCH, 128, kp).transpose(1, 0, 2).astype(f16)
        )
        wq = Wqkv[:, h0 * D : (h0 + HC) * D]
        wq = np.ascontiguousarray(
            wq.reshape(KCH, 128, 3, 128).transpose(1, 0, 2, 3).astype(f16)
        )
        wkk = Wqkv[:, E + h0 * D : E + (h0 + HC) * D]
        wkk = np.ascontiguousarray(
            wkk.reshape(KCH, 128, 3, 128).transpose(1, 0, 2, 3).astype(f16)
        )
        wvv = Wqkv[:, 2 * E + h0 * D : 2 * E + (h0 + HC) * D]
        wvv = np.ascontiguousarray(
            wvv.reshape(KCH, 128, VC).transpose(1, 0, 2).astype(f16)
        )
        wpp = Wproj[hg * VC : (hg + 1) * VC, :]
        wpp = np.ascontiguousarray(
            wpp.reshape(3, 128, E).transpose(1, 0, 2).astype(f16)
        )
        bqq = np.ascontiguousarray(
            bqkv[h0 * D : (h0 + HC) * D].reshape(3, 128).T.astype(np.float32)
        )
        bkk = np.ascontiguousarray(
            bqkv[E + h0 * D : E + (h0 + HC) * D]
            .reshape(3, 128).T.astype(np.float32)
        )
        keepmask = (np.arange(kp) < kept).astype(f16).reshape(nkc, 128).T
        onesr = np.ascontiguousarray(
            np.repeat(keepmask[:, :, None], HC, axis=2).astype(f16)
        )
        maps.append(
            {
                "xt": xt4, "xtk": xtk, "wq": wq, "wk": wkk, "wv": wvv,
                "wp": wpp, "bq": bqq, "bk": bkk, "ones": onesr,
            }
        )
    return maps


def run(x, mask, Wqkv, bqkv, Wproj, bproj, trace=False, trace_cores=None):
    mask = np.asarray(mask)
    Wproj_np = np.asarray(Wproj, np.float32)
    bproj_np = np.asarray(bproj, np.float32)
    bqkv_np = np.asarray(bqkv, np.float32)
    kept = (mask[:, 0, 0, :] != 0).sum(axis=1)
    kp = max(128, int(-(-kept.max() // 128)) * 128)
    in_maps = make_core_inputs(x, mask, Wqkv, bqkv_np, Wproj_np, kp)

    nc = build_program(kp)
    try:
        res = run_bass_kernel_spmd(
            nc, in_maps, core_ids=list(range(8)), trace=trace,
            trace_cores=trace_cores,
        )
    except Exception:
        # transient device wedge -- one retry is usually enough
        res = run_bass_kernel_spmd(
            nc, in_maps, core_ids=list(range(8)), trace=trace,
            trace_cores=trace_cores,
        )

    # host-folded bias: v-bias passes through softmax (weights sum to 1)
    bv = bqkv_np[2 * E : 3 * E]
    bias_row = bv @ Wproj_np + bproj_np
    y = np.empty((B, S, E), np.float32)
    for b in range(B):
        p0 = res.results[2 * b]["y"].reshape(E, S).astype(np.float32)
        p1 = res.results[2 * b + 1]["y"].reshape(E, S).astype(np.float32)
        y[b] = p0.T + p1.T + bias_row
    return y, res


def kernel(x, mask, Wqkv, bqkv, Wproj, bproj):
    y, _ = run(x, mask, Wqkv, bqkv, Wproj, bproj, trace=False)
    return y


# revision 74
# speedup vs baseline: 1.0173x; 1.0023x over previous
"""Trainium2 Bass kernel for CodeAttention (B=4, S=2048, E=768, H=12).

Sharding: 8 cores = 4 batches x 2 head-groups (6 heads each). Each core
computes a partial projection output for its batch; the host sums the two
partials per batch and adds the (host-folded) bias row.

Design (fp16 datapath; ~149us/core cost-model estimate vs the 270us
fused baseline; max rel err ~6.7e-4):
- Key compaction: the padding mask is known on the host, so masked keys
  (~50%) are gathered OUT of the K/V stream entirely (exact math: they
  contribute to neither the numerator nor the softmax denominator). Kept
  keys are padded to KP (multiple of 128) with zero columns whose ones-
  column entry is 0, which keeps them exactly inert.
- pv orientation flip: out[q,65] = sum_k pt[k,q]*vst[k,65] makes the
  moving operand the 65-wide V tile, cutting pv PE rows ~2x vs moving
  the 512-wide query block. The 65th column accumulates the softmax
  denominator, so normalization is a per-partition reciprocal+scale on
  DVE (no gpsimd broadcast). Two query-chunk accumulators share each
  PSUM bank under a single start/stop group.
- x arrives pre-transposed from the host (xt, xtk), so there are no
  on-chip x transposes; att is re-transposed on PE (48 small transposes)
  for the output projection, and y leaves as yT (host re-transposes).
- Main rhythm: per (query-half, head) 9 key-chunk slots: exp(kc) on ACT,
  then scores(kc+1) (one slot of lookahead keeps ACT fed -- cross-engine
  deps are emission-ordered counters), then pv groups of the previous
  head, then statically scheduled filler units (q/k/v/proj/attT) spread
  so no head period overloads PE (v is head-pair granular).
- The modeled DMA bus is near serial: all input loads ride one queue in
  exact priority order (first-exp critical path leads); the first two
  exps are split into 512-wide halves so ACT starts before xt-sb1 lands.
- Last head: its pv pairs accumulate DURING its own slots (un ring +
  drained pv ring), so the tail is norms -> attT dt2 -> output
  projections, with the final stores batched into 3-tile DMAs.
"""

import sys

if "/opt/trn_rl_repo" not in sys.path:
    sys.path.insert(0, "/opt/trn_rl_repo")

import numpy as np

import concourse.bass as bass  # noqa: F401
import concourse.mybir as mybir
import concourse.tile as tile
from concourse import bacc
from concourse.alu_op_type import AluOpType
from concourse.bass_utils import run_bass_kernel_spmd
from concourse.masks import make_identity

F32 = mybir.dt.float32
F32R = mybir.dt.float32r
FP16 = mybir.dt.float16
Act = mybir.ActivationFunctionType

B, S, E, H, D = 4, 2048, 768, 12, 64
HC = 6                    # heads per core
KCH = E // 128            # contraction chunks over E = 6
VC = HC * D               # v columns per core = 384
VW = D + 1                # v width incl. ones column = 65
DEFAULT_KP = 1152         # padded kept-key count for the fixed-seed mask


def build_program(kp=DEFAULT_KP):
    nkc = kp // 128
    nc = bacc.Bacc("TRN2", target_bir_lowering=False, debug=False, num_devices=8)

    xt_d = nc.dram_tensor("xt", [4, 128, KCH, 512], FP16, kind="ExternalInput")
    xtk_d = nc.dram_tensor("xtk", [128, KCH, kp], FP16, kind="ExternalInput")
    wq_d = nc.dram_tensor("wq", [128, KCH, 3, 128], FP16, kind="ExternalInput")
    wk_d = nc.dram_tensor("wk", [128, KCH, 3, 128], FP16, kind="ExternalInput")
    wv_d = nc.dram_tensor("wv", [128, KCH, VC], FP16, kind="ExternalInput")
    wp_d = nc.dram_tensor("wp", [128, 3, E], FP16, kind="ExternalInput")
    bq_d = nc.dram_tensor("bq", [128, 3], F32, kind="ExternalInput")
    bk_d = nc.dram_tensor("bk", [128, 3], F32, kind="ExternalInput")
    ones_d = nc.dram_tensor("ones", [128, nkc, HC], FP16, kind="ExternalInput")
    y_d = nc.dram_tensor("y", [HC, 128, S], FP16, kind="ExternalOutput")

    with tile.TileContext(nc) as tc:
        _emit(nc, tc, nkc, xt_d, xtk_d, wq_d, wk_d, wv_d, wp_d, bq_d, bk_d,
              ones_d, y_d)
    nc.compile()
    return nc


def _build_schedule(nkc):
    """slot -> list of filler units. Slots are (qbp, h, kc) flattened.

    Units: ("q", sb, m), ("k", m, kb), ("v", kc), ("at", qc8, dt, qbp),
    ("pj", Et, qb). Placement rules keep each unit >= a few slots ahead
    of its first consumer (see design notes in the module docstring).
    """
    fillers = {}

    def put(qbp, h, kc, u):
        i = (qbp * HC + h) * nkc + min(kc, nkc - 1)
        fillers.setdefault(i, []).append(u)

    # v units, head-pair granular: pair p needed by pv(h=2p) which runs
    # during head 2p+1; spread them so no single head period overloads PE
    for kc in range(nkc):
        put(0, 0, kc, ("v", kc, 0))
        put(0, 2, kc, ("v", kc, 1))
        put(0, 4, kc, ("v", kc, 2))
    # k units (prologue does m0 kb0 only); m-tile m needed by heads 2m..;
    # kb block j only feeds score chunks kc >= 4j, so later blocks are JIT
    nkb = (nkc + 3) // 4
    for j in range(1, nkb):
        put(0, 0, 2 * j - 1, ("k", 0, j))
    put(0, 1, 0, ("k", 1, 0))
    put(0, 1, 6, ("k", 1, 1))
    put(0, 2, 1, ("k", 1, 2))
    put(0, 3, 5, ("k", 2, 0))
    put(0, 3, 7, ("k", 2, 1))
    put(0, 4, 1, ("k", 2, 2))
    # q units (prologue does sb0/sb1 m0); m-tile m needed by heads 2m
    put(0, 1, 2, ("q", 0, 1))
    put(0, 1, 4, ("q", 1, 1))
    put(0, 3, 1, ("q", 0, 2))
    put(0, 3, 3, ("q", 1, 2))
    put(0, 5, 1, ("q", 2, 0))
    put(0, 5, 3, ("q", 3, 0))
    put(1, 0, 1, ("q", 2, 1))
    put(1, 0, 3, ("q", 3, 1))
    put(1, 1, 1, ("q", 2, 2))
    put(1, 1, 3, ("q", 3, 2))
    # attT transposes: (qc8, dt) one head-period after norm(2dt+1, qc8).
    # dt1 of the second query half moves INTO (1,4): its pv-pair norms
    # land at slot qc8//2 there, and (1,5) must keep the un ring free for
    # the tracking accumulators (and its sc ring free for scores).
    for qc8 in range(8):
        put(0, 3, qc8, ("at", qc8, 0, 0))
        put(0, 5, qc8, ("at", qc8, 1, 0))
        put(1, 1, qc8, ("at", qc8, 2, 0))
        put(1, 3, qc8, ("at", qc8, 0, 1))
        put(1, 4, qc8 // 2 + 1, ("at", qc8, 1, 1))
    # proj qb0/qb1 spread through the (light) qbp1 head periods
    for Et in range(6):
        if Et < 3:
            put(1, 1, 5 + Et, ("pj", Et, 0))
        else:
            put(1, 2, 2 * (Et - 3) + 1, ("pj", Et, 0))
        if Et < 4:
            put(1, 3, 2 * Et + 1, ("pj", Et, 1))
        else:
            put(1, 4, 2 * (Et - 4) + 1, ("pj", Et, 1))
    return fillers


def _emit(nc, tc, nkc, xt_d, xtk_d, wq_d, wk_d, wv_d, wp_d, bq_d, bk_d,
          ones_d, y_d):
    kp = nkc * 128
    nkb = (kp + 511) // 512  # k-unit key blocks (512-wide, last ragged)
    ctx_pools = []

    def pool(name, bufs, space="SBUF"):
        p = tc.tile_pool(name=name, bufs=bufs, space=space)
        ctx_pools.append(p)
        return p.__enter__()

    consts = pool("consts", 1)
    store = pool("store", 1)
    pt_p = pool("pt", 2)
    sc_p = pool("sc", 2, space="PSUM")    # [128,1024] f32 = 2 banks each
    pv_p = pool("pv", 2, space="PSUM")    # [128,512] f32 = 1 bank each
    un_p = pool("un", 2, space="PSUM")    # [128,512] f32 = 1 bank each
    ys_p = pool("ys", 4)
    rs_p = pool("rs", 2)

    ident = consts.tile([128, 128], FP16)
    wq = consts.tile([128, KCH, 3, 128], FP16)
    wk = consts.tile([128, KCH, 3, 128], FP16)
    wv = consts.tile([128, KCH, VC], FP16)
    wp = consts.tile([128, 3, E], FP16)
    bq = consts.tile([128, 3], F32)
    bk = consts.tile([128, 3], F32)
    ones = consts.tile([128, nkc, HC], FP16)

    # The modeled DMA bus is near serial and only per-queue FIFO order is
    # controllable (SWDGE desc-gen has no waits, so it races the bus), so
    # ALL input loads go on the sync queue in exact priority order: the
    # critical path to the first exp (wk m0, xtk c0, wq m0, xt sb0/sb1)
    # first, then everything else by first use.


    xts = store.tile([128, KCH, S], FP16, name="xts")
    xtk = store.tile([128, KCH, kp], FP16, name="xtk")
    qT = [store.tile([128, 3, 512], FP16, name=f"qT{sb}") for sb in range(4)]
    kT = store.tile([128, 3, kp], FP16, name="kT")
    vst = store.tile([128, nkc, HC, VW], FP16, name="vst")
    att = store.tile([128, 16, VC], FP16, name="att")
    attT = store.tile([128, 3, S], FP16, name="attT")

    # x loads on the sync queue, halves first so q-unit matmuls can start
    # as soon as the first three contraction chunks land
    def load_xt(sb):
        for half in range(2):
            ks = slice(3 * half, 3 * half + 3)
            nc.sync.dma_start(
                xts[:, ks, sb * 512 : (sb + 1) * 512], xt_d.ap()[sb][:, ks, :]
            )

    def load_xtk(c0, c1):
        nc.sync.dma_start(xtk[:, :, c0:c1], xtk_d.ap()[:, :, c0:c1])

    nc.sync.dma_start(wk[:, :, 0, :], wk_d.ap()[:, :, 0, :])
    # first key block in contraction-halves so the first k-unit matmuls
    # start one transfer earlier
    nc.sync.dma_start(xtk[:, 0:3, 0:512], xtk_d.ap()[:, 0:3, 0:512])
    nc.sync.dma_start(xtk[:, 3:6, 0:512], xtk_d.ap()[:, 3:6, 0:512])
    nc.sync.dma_start(bk[:], bk_d.ap())
    nc.sync.dma_start(wq[:, :, 0, :], wq_d.ap()[:, :, 0, :])
    nc.sync.dma_start(bq[:], bq_d.ap())
    load_xt(0)
    load_xt(1)
    nc.sync.dma_start(ones[:], ones_d.ap())
    nc.sync.dma_start(wv[:], wv_d.ap())
    make_identity(nc, ident[:])
    if kp > 512:
        load_xtk(512, min(kp, 1024))
    nc.sync.dma_start(wq[:, :, 1:3, :], wq_d.ap()[:, :, 1:3, :])
    nc.sync.dma_start(wk[:, :, 1:3, :], wk_d.ap()[:, :, 1:3, :])
    if kp > 1024:
        load_xtk(1024, kp)
    load_xt(2)
    nc.sync.dma_start(wp[:], wp_d.ap())
    load_xt(3)

    # ---- units -----------------------------------------------------------
    def unit_q(sb, m):
        u = un_p.tile([128, 512], F32, tag="un", name=f"uq{sb}_{m}")
        for k in range(KCH):
            nc.tensor.matmul(
                u[:], wq[:, k, m, :], xts[:, k, sb * 512 : (sb + 1) * 512],
                start=(k == 0), stop=(k == KCH - 1),
            )
        nc.vector.tensor_scalar_add(qT[sb][:, m, :], u[:], bq[:, m : m + 1])

    def unit_k(m, kb):
        c0, c1 = kb * 512, min((kb + 1) * 512, kp)
        u = un_p.tile([128, 512], F32, tag="un", name=f"uk{m}_{kb}")
        for k in range(KCH):
            nc.tensor.matmul(
                u[:, 0 : c1 - c0], wk[:, k, m, :], xtk[:, k, c0:c1],
                start=(k == 0), stop=(k == KCH - 1),
            )
        nc.vector.tensor_scalar_add(
            kT[:, m, c0:c1], u[:, 0 : c1 - c0], bk[:, m : m + 1]
        )

    def unit_v(kc, p):
        # one head-pair's v columns: keeps the v work out of the first
        # head period (pv of head h only needs pair h//2's columns)
        u = un_p.tile([128, 512], F32, tag="un", name=f"uv{kc}_{p}")
        for k in range(KCH):
            nc.tensor.matmul(
                u[:, 0:128], xtk[:, k, kc * 128 : (kc + 1) * 128],
                wv[:, k, p * 128 : (p + 1) * 128],
                start=(k == 0), stop=(k == KCH - 1),
            )
        nc.vector.tensor_copy(
            vst[:, kc, 2 * p : 2 * p + 2, 0:D],
            u[:, 0:128].rearrange("p (h d) -> p h d", h=2),
        )
        nc.vector.tensor_copy(
            vst[:, kc, 2 * p : 2 * p + 2, D : D + 1],
            ones[:, kc : kc + 1, 2 * p : 2 * p + 2].rearrange(
                "p one h -> p h one"
            ),
        )

    def unit_attT(qc8, dt, qbp, pool=None):
        qc = qbp * 8 + qc8
        tr = (pool or un_p).tile([128, 128], FP16,
                                 tag="sc" if pool is sc_p else "un",
                                 name=f"tr{qc}_{dt}")
        nc.tensor.matmul(
            tr[:], att[:, qc, dt * 128 : (dt + 1) * 128], ident[:],
            is_transpose=True, start=True, stop=True,
        )
        nc.vector.tensor_copy(attT[:, dt, qc * 128 : (qc + 1) * 128], tr[:])

    def unit_proj(Et, qb, pool=None, evac=None):
        u = (pool or un_p).tile([128, 512], F32,
                                tag="pv" if pool is pv_p else "un",
                                name=f"up{Et}_{qb}")
        for dt in range(3):
            nc.tensor.matmul(
                u[:], wp[:, dt, Et * 128 : (Et + 1) * 128],
                attT[:, dt, qb * 512 : (qb + 1) * 512],
                start=(dt == 0), stop=(dt == 2),
            )
        ys = ys_p.tile([128, 512], FP16, tag="ys", name="ys")
        if evac is nc.scalar:
            nc.scalar.copy(ys[:], u[:])
            # keep the y-store config off the ACT SEQ (it would serialize
            # with the evacuation copies)
            nc.sync.dma_start(y_d.ap()[Et][:, qb * 512 : (qb + 1) * 512],
                              ys[:])
        else:
            nc.vector.tensor_copy(ys[:], u[:])
            eng = nc.sync if (Et + qb) % 2 == 0 else nc.scalar
            eng.dma_start(y_d.ap()[Et][:, qb * 512 : (qb + 1) * 512], ys[:])

    def pv_mms(acc, pt, h, qcs, kcs, start, stop):
        n = len(qcs)
        for ki, kc in enumerate(kcs):
            for x, qc in enumerate(qcs):
                nc.tensor.matmul(
                    acc[:, x * VW : (x + 1) * VW],
                    pt[:, kc, (qc % 8) * 128 : (qc % 8 + 1) * 128],
                    vst[:, kc, h, :],
                    start=(start and ki == 0 and x == 0),
                    stop=(stop and ki == len(kcs) - 1 and x == n - 1),
                )

    def pv_norms(acc, h, qcs):
        for x, qc in enumerate(qcs):
            rse = rs_p.tile([128, 1], F32, tag="rs", name="rse")
            with nc.allow_low_precision(reason="f32r is full width"):
                nc.vector.reciprocal(rse[:], acc[:, x * VW + D : x * VW + D + 1])
            nc.vector.tensor_scalar_mul(
                att[:, qc, h * D : (h + 1) * D],
                acc[:, x * VW : x * VW + D], rse[:],
            )

    def pv_group(pt, h, qcs):
        # one PSUM bank accumulates len(qcs) (<=2) query chunks: a single
        # start/stop accumulation group, halving pv ring turnover
        acc = pv_p.tile([128, 512], F32, tag="pv", name=f"pv{qcs[0]}_{h}")
        pv_mms(acc, pt, h, qcs, range(nkc), True, True)
        pv_norms(acc, h, qcs)

    def emit_unit(u):
        kind = u[0]
        if kind == "q":
            unit_q(u[1], u[2])
        elif kind == "k":
            unit_k(u[1], u[2])
        elif kind == "v":
            unit_v(u[1], u[2])
        elif kind == "at":
            unit_attT(u[1], u[2], u[3])
        elif kind == "pj":
            unit_proj(u[1], u[2])

    # ---- prologue units --------------------------------------------------
    unit_k(0, 0)
    unit_q(0, 0)

    fillers = _build_schedule(nkc)
    nslots = 2 * HC * nkc
    sc_pending = []  # score tiles awaiting their exp, FIFO

    def scores_mm(st, flat, j):
        qbp, rem = divmod(flat, HC * nkc)
        h, kc = divmod(rem, nkc)
        hp, r0 = h // 2, (h % 2) * 64
        sb = 2 * qbp + j
        nc.tensor.matmul(
            st[:, j * 512 : (j + 1) * 512],
            kT[r0 : r0 + 64, hp, kc * 128 : (kc + 1) * 128],
            qT[sb][r0 : r0 + 64, hp, :],
            start=True, stop=True,
        )

    def emit_scores(flat):
        st = sc_p.tile([128, 1024], F32, tag="sc", name="st")
        scores_mm(st, flat, 0)
        scores_mm(st, flat, 1)
        sc_pending.append(st)

    # front pipeline: the j0 halves of the first two score chunks depend
    # only on the sb0 query block (early on the DMA bus); q(1,0) and the
    # j1 halves follow once sb1 lands. Cross-engine deps are emission-
    # ordered, so this ordering is what lets ACT start ~2us earlier.
    if nkc >= 2:
        st0 = sc_p.tile([128, 1024], F32, tag="sc", name="st0")
        st1 = sc_p.tile([128, 1024], F32, tag="sc", name="st1")
        scores_mm(st0, 0, 0)
        scores_mm(st1, 1, 0)
    else:
        unit_q(1, 0)
        emit_scores(0)

    # ---- main loop: exp(i) -> scores(i+1) -> pv -> fillers ---------------
    pt_prev = None  # pt tile whose pv groups run during this head period
    pt_cur = None
    # last head: its pv pairs accumulate DURING its own slots ("tracking"),
    # using the idle un ring for pairs 0/1 and the pv ring (as its previous
    # user drains) for pairs 2/3 -- the tail then starts at norms directly
    track = [None] * 4
    # pair -> first emission slot; tracking runs ONE chunk behind the exp
    # stream (8 waiting matmuls would overflow the 4-deep bypass window
    # and stall PE in-order behind the in-flight exp)
    talloc = {0: 1, 1: 1, 2: 5, 3: 6}
    for qbp in range(2):
        for h in range(HC):
            tracking = qbp == 1 and h == HC - 1 and nkc >= 6
            pt_prev = pt_cur
            pt_cur = pt_p.tile([128, nkc, 1024], FP16, tag="pt",
                               name=f"pt{qbp}_{h}")
            for kc in range(nkc):
                i = (qbp * HC + h) * nkc + kc
                if i == 0 and nkc >= 2:
                    # first two slots: exp in 512-wide halves, j0 halves
                    # first -- the sb0 query block lands well before sb1 on
                    # the serialized DMA bus, so ACT starts ~4us earlier
                    for stx, kx in ((st0, 0), (st1, 1)):
                        nc.scalar.activation(
                            pt_cur[:, kx, 0:512], stx[:, 0:512], Act.Exp,
                            scale=0.125,
                        )
                    unit_q(1, 0)
                    scores_mm(st0, 0, 1)
                    scores_mm(st1, 1, 1)
                    emit_scores(2)
                    for stx, kx in ((st0, 0), (st1, 1)):
                        nc.scalar.activation(
                            pt_cur[:, kx, 512:1024], stx[:, 512:1024],
                            Act.Exp, scale=0.125,
                        )
                    if nkc >= 4:
                        emit_scores(3)
                    for u in fillers.get(0, ()):
                        emit_unit(u)
                    continue
                if i == 1 and nkc >= 2:
                    for u in fillers.get(1, ()):
                        emit_unit(u)
                    continue
                st = sc_pending.pop(0)
                nc.scalar.activation(pt_cur[:, kc, :], st[:], Act.Exp,
                                     scale=0.125)
                # two-slot lookahead: tile (i+2)%2 is freed by this exp,
                # and heavy filler slots can no longer delay the next-next
                # score chunk (ACT stays fed through PE-overloaded periods)
                la = i + 2 if nkc >= 4 else i + 1
                if la < nslots and (nkc < 4 or la >= 4):
                    emit_scores(la)
                if pt_prev is not None:
                    # pair j of the previous head runs at slot j,
                    # clamped into this head period
                    pvs = []
                    for j in range(4):
                        if min(j, nkc - 1) == kc:
                            pvs += [2 * j, 2 * j + 1]
                    ph = (h - 1) % HC
                    pqbp = qbp if h > 0 else qbp - 1
                    for x in range(0, len(pvs), 2):
                        pv_group(pt_prev, ph,
                                 [pqbp * 8 + q for q in pvs[x : x + 2]])
                if tracking:
                    for j in range(4):
                        k0 = talloc[j]
                        if kc < k0:
                            continue
                        qcs = [8 + 2 * j, 9 + 2 * j]
                        if kc == k0:
                            pl, tg = (un_p, "un") if j < 2 else (pv_p, "pv")
                            track[j] = pl.tile([128, 512], F32, tag=tg,
                                               name=f"tk{j}")
                            pv_mms(track[j], pt_cur, h, qcs,
                                   range(0, kc), True, False)
                        else:
                            pv_mms(track[j], pt_cur, h, qcs, [kc - 1],
                                   False, False)
                for u in fillers.get(i, ()):
                    if tracking and u[0] == "at":
                        unit_attT(u[1], u[2], u[3], pool=sc_p)
                    else:
                        emit_unit(u)

    # ---- tail: last head's pv + attT dt2 + proj qb2/qb3 ------------------
    # interleave so PE never sits on the pv->norm->attT DVE chains: each
    # attT lags its pv by one group, proj units weave between pv groups as
    # soon as their four attT columns are present.
    if nkc >= 6:
        # finish the tracked accumulators (last key chunk) right after the
        # final exp; the tail is then norms -> attT dt2 -> proj qb2/qb3,
        # with the proj units rotating over both freed PSUM rings
        for j in range(4):
            pv_mms(track[j], pt_cur, HC - 1, [8 + 2 * j, 9 + 2 * j],
                   [nkc - 1], False, True)
        pv_norms(track[0], HC - 1, [8, 9])
        pv_norms(track[1], HC - 1, [10, 11])
        for qc8 in range(4):
            unit_attT(qc8, 2, 1, pool=sc_p)
        pv_norms(track[2], HC - 1, [12, 13])
        pv_norms(track[3], HC - 1, [14, 15])
        ys2 = store.tile([128, HC, 512], FP16, name="ys2")

        def proj_qb2(Et, pl):
            u = pl.tile([128, 512], F32, tag="pv" if pl is pv_p else "un",
                        name=f"up{Et}_2")
            for dt in range(3):
                nc.tensor.matmul(
                    u[:], wp[:, dt, Et * 128 : (Et + 1) * 128],
                    attT[:, dt, 2 * 512 : 3 * 512],
                    start=(dt == 0), stop=(dt == 2),
                )
            if Et % 2 == 0:
                nc.scalar.copy(ys2[:, Et, :], u[:])
            else:
                nc.vector.tensor_copy(ys2[:, Et, :], u[:])
            if Et == 2:
                nc.sync.dma_start(
                    y_d.ap()[0:3, :, 2 * 512 : 3 * 512].rearrange(
                        "e p s -> p e s"
                    ),
                    ys2[:, 0:3, :],
                )
            elif Et == 5:
                nc.sync.dma_start(
                    y_d.ap()[3:6, :, 2 * 512 : 3 * 512].rearrange(
                        "e p s -> p e s"
                    ),
                    ys2[:, 3:6, :],
                )

        proj_qb2(0, un_p)
        proj_qb2(1, un_p)
        for qc8 in range(4, 8):
            unit_attT(qc8, 2, 1, pool=sc_p)
        for Et, pl in ((2, pv_p), (3, pv_p), (4, un_p), (5, un_p)):
            proj_qb2(Et, pl)
        # final query block: evacuate into one staging tile and store in
        # two 3-tile DMAs -- per-store HWDGE configs would serialize the
        # end-of-kernel chain
        ys6 = store.tile([128, HC, 512], FP16, name="ys6")
        for Et, pl in ((0, pv_p), (1, pv_p), (2, un_p), (3, un_p),
                       (4, pv_p), (5, pv_p)):
            u = pl.tile([128, 512], F32, tag="pv" if pl is pv_p else "un",
                        name=f"up{Et}_3")
            for dt in range(3):
                nc.tensor.matmul(
                    u[:], wp[:, dt, Et * 128 : (Et + 1) * 128],
                    attT[:, dt, 3 * 512 : 4 * 512],
                    start=(dt == 0), stop=(dt == 2),
                )
            if Et % 2 == 0:
                nc.scalar.copy(ys6[:, Et, :], u[:])
            else:
                nc.vector.tensor_copy(ys6[:, Et, :], u[:])
            if Et == 2:
                nc.sync.dma_start(
                    y_d.ap()[0:3, :, 3 * 512 : 4 * 512].rearrange(
                        "e p s -> p e s"
                    ),
                    ys6[:, 0:3, :],
                )
            elif Et == 4:
                nc.sync.dma_start(
                    y_d.ap()[3:5, :, 3 * 512 : 4 * 512].rearrange(
                        "e p s -> p e s"
                    ),
                    ys6[:, 3:5, :],
                )
        # the very last store is a single tile so the end-of-kernel chain
        # rides the shortest possible transfer
        nc.sync.dma_start(
            y_d.ap()[5:6, :, 3 * 512 : 4 * 512].rearrange("e p s -> p e s"),
            ys6[:, 5:6, :],
        )
    else:
        for pp in range(4):
            pv_group(pt_cur, HC - 1, [8 + 2 * pp, 9 + 2 * pp])
            if pp >= 1:
                unit_attT(2 * pp - 2, 2, 1)
                unit_attT(2 * pp - 1, 2, 1)
        unit_attT(6, 2, 1)
        unit_attT(7, 2, 1)
        for Et in range(6):
            unit_proj(Et, 2)
        for Et in range(6):
            unit_proj(Et, 3)

    for p in reversed(ctx_pools):
        p.__exit__(None, None, None)


def make_core_inputs(x, mask, Wqkv, bqkv, Wproj, kp):
    """Slice full inputs into 8 per-core input maps (host-side layouts)."""
    x = np.asarray(x, np.float32)
    mask = np.asarray(mask)
    Wqkv = np.asarray(Wqkv, np.float32)
    bqkv = np.asarray(bqkv, np.float32)
    Wproj = np.asarray(Wproj, np.float32)
    nkc = kp // 128
    f16 = np.float16
    maps = []
    for c in range(8):
        b, hg = c // 2, c % 2
        h0 = hg * HC
        keep = np.nonzero(mask[b, 0, 0, :] != 0)[0]
        kept = len(keep)
        xt = x[b].T.reshape(KCH, 128, S).transpose(1, 0, 2)  # [p, kch, s]
        xt4 = np.ascontiguousarray(
            xt.reshape(128, KCH, 4, 512).transpose(2, 0, 1, 3).astype(f16)
        )
        xk = np.zeros((kp, E), np.float32)
        xk[:kept] = x[b, keep, :]
        xtk = np.ascontiguousarray(
            xk.T.reshape(KCH, 128, kp).transpose(1, 0, 2).astype(f16)
        )
        wq = Wqkv[:, h0 * D : (h0 + HC) * D]
        wq = np.ascontiguousarray(
            wq.reshape(KCH, 128, 3, 128).transpose(1, 0, 2, 3).astype(f16)
        )
        wkk = Wqkv[:, E + h0 * D : E + (h0 + HC) * D]
        wkk = np.ascontiguousarray(
            wkk.reshape(KCH, 128, 3, 128).transpose(1, 0, 2, 3).astype(f16)
        )
        wvv = Wqkv[:, 2 * E + h0 * D : 2 * E + (h0 + HC) * D]
        wvv = np.ascontiguousarray(
            wvv.reshape(KCH, 128, VC).transpose(1, 0, 2).astype(f16)
        )
        wpp = Wproj[hg * VC : (hg + 1) * VC, :]
        wpp = np.ascontiguousarray(
            wpp.reshape(3, 128, E).transpose(1, 0, 2).astype(f16)
        )
        bqq = np.ascontiguousarray(
            bqkv[h0 * D : (h0 + HC) * D].reshape(3, 128).T.astype(np.float32)
        )
        bkk = np.ascontiguousarray(
            bqkv[E + h0 * D : E + (h0 + HC) * D]
            .reshape(3, 128).T.astype(np.float32)
        )
        keepmask = (np.arange(kp) < kept).astype(f16).reshape(nkc, 128).T
        onesr = np.ascontiguousarray(
            np.repeat(keepmask[:, :, None], HC, axis=2).astype(f16)
        )
        maps.append(
            {
                "xt": xt4, "xtk": xtk, "wq": wq, "wk": wkk, "wv": wvv,
                "wp": wpp, "bq": bqq, "bk": bkk, "ones": onesr,
            }
        )
    return maps


def run(x, mask, Wqkv, bqkv, Wproj, bproj, trace=False, trace_cores=None):
    mask = np.asarray(mask)
    Wproj_np = np.asarray(Wproj, np.float32)
    bproj_np = np.asarray(bproj, np.float32)
    bqkv_np = np.asarray(bqkv, np.float32)
    kept = (mask[:, 0, 0, :] != 0).sum(axis=1)
    kp = max(128, int(-(-kept.max() // 128)) * 128)
    in_maps = make_core_inputs(x, mask, Wqkv, bqkv_np, Wproj_np, kp)

    nc = build_program(kp)
    try:
        res = run_bass_kernel_spmd(
            nc, in_maps, core_ids=list(range(8)), trace=trace,
            trace_cores=trace_cores,
        )
    except Exception:
        # transient device wedge -- one retry is usually enough
        res = run_bass_kernel_spmd(
            nc, in_maps, core_ids=list(range(8)), trace=trace,
            trace_cores=trace_cores,
        )

    # host-folded bias: v-bias passes through softmax (weights sum to 1)
    bv = bqkv_np[2 * E : 3 * E]
    bias_row = bv @ Wproj_np + bproj_np
    y = np.empty((B, S, E), np.float32)
    for b in range(B):
        p0 = res.results[2 * b]["y"].reshape(E, S).astype(np.float32)
        p1 = res.results[2 * b + 1]["y"].reshape(E, S).astype(np.float32)
        y[b] = p0.T + p1.T + bias_row
    return y, res


def kernel(x, mask, Wqkv, bqkv, Wproj, bproj):
    y, _ = run(x, mask, Wqkv, bqkv, Wproj, bproj, trace=False)
    return y


# revision 80
# speedup vs baseline: 1.0194x; 1.0021x over previous
"""Trainium2 Bass kernel for CodeAttention (B=4, S=2048, E=768, H=12).

Sharding: 8 cores = 4 batches x 2 head-groups (6 heads each). Each core
computes a partial projection output for its batch; the host sums the two
partials per batch and adds the (host-folded) bias row.

Design (fp16 datapath; ~146.5us/core cost-model estimate vs the 270us
fused baseline; max rel err ~6.7e-4):
- Key compaction: the padding mask is known on the host, so masked keys
  (~50%) are gathered OUT of the K/V stream entirely (exact math: they
  contribute to neither the numerator nor the softmax denominator). Kept
  keys are padded to KP (multiple of 128) with zero columns whose ones-
  column entry is 0, which keeps them exactly inert.
- pv orientation flip: out[q,65] = sum_k pt[k,q]*vst[k,65] makes the
  moving operand the 65-wide V tile, cutting pv PE rows ~2x vs moving
  the 512-wide query block. The 65th column accumulates the softmax
  denominator, so normalization is a per-partition reciprocal+scale on
  DVE (no gpsimd broadcast). Two query-chunk accumulators share each
  PSUM bank under a single start/stop group.
- x arrives pre-transposed from the host (xt, xtk), so there are no
  on-chip x transposes; att is re-transposed on PE (48 small transposes)
  for the output projection, and y leaves as yT (host re-transposes).
- Main rhythm: per (query-half, head) 9 key-chunk slots: exp(kc) on ACT,
  then scores(kc+1) (one slot of lookahead keeps ACT fed -- cross-engine
  deps are emission-ordered counters), then pv groups of the previous
  head, then statically scheduled filler units (q/k/v/proj/attT) spread
  so no head period overloads PE (v is head-pair granular).
- The modeled DMA bus is near serial: all input loads ride one queue in
  exact priority order (first-exp critical path leads); the first two
  exps are split into 512-wide halves so ACT starts before xt-sb1 lands.
- Last head: its pv pairs accumulate DURING its own slots (un ring +
  drained pv ring), so the tail is norms -> attT dt2 -> output
  projections, with the final stores batched into 3-tile DMAs.
"""

import sys

if "/opt/trn_rl_repo" not in sys.path:
    sys.path.insert(0, "/opt/trn_rl_repo")

import numpy as np

import concourse.bass as bass  # noqa: F401
import concourse.mybir as mybir
import concourse.tile as tile
from concourse import bacc
from concourse.alu_op_type import AluOpType
from concourse.bass_utils import run_bass_kernel_spmd
from concourse.masks import make_identity

F32 = mybir.dt.float32
F32R = mybir.dt.float32r
FP16 = mybir.dt.float16
Act = mybir.ActivationFunctionType

B, S, E, H, D = 4, 2048, 768, 12, 64
HC = 6                    # heads per core
KCH = E // 128            # contraction chunks over E = 6
VC = HC * D               # v columns per core = 384
VW = D + 1                # v width incl. ones column = 65
DEFAULT_KP = 1152         # padded kept-key count for the fixed-seed mask


def build_program(kp=DEFAULT_KP):
    nkc = kp // 128
    nc = bacc.Bacc("TRN2", target_bir_lowering=False, debug=False, num_devices=8)

    xt_d = nc.dram_tensor("xt", [4, 128, KCH, 512], FP16, kind="ExternalInput")
    xtk_d = nc.dram_tensor("xtk", [128, KCH, kp], FP16, kind="ExternalInput")
    wq_d = nc.dram_tensor("wq", [128, KCH, 3, 128], FP16, kind="ExternalInput")
    wk_d = nc.dram_tensor("wk", [128, KCH, 3, 128], FP16, kind="ExternalInput")
    wv_d = nc.dram_tensor("wv", [128, KCH, VC], FP16, kind="ExternalInput")
    wp_d = nc.dram_tensor("wp", [128, 3, E], FP16, kind="ExternalInput")
    bq_d = nc.dram_tensor("bq", [128, 3], F32, kind="ExternalInput")
    bk_d = nc.dram_tensor("bk", [128, 3], F32, kind="ExternalInput")
    ones_d = nc.dram_tensor("ones", [128, nkc, HC], FP16, kind="ExternalInput")
    y_d = nc.dram_tensor("y", [HC, 128, S], FP16, kind="ExternalOutput")

    with tile.TileContext(nc) as tc:
        _emit(nc, tc, nkc, xt_d, xtk_d, wq_d, wk_d, wv_d, wp_d, bq_d, bk_d,
              ones_d, y_d)
    nc.compile()
    return nc


def _build_schedule(nkc):
    """slot -> list of filler units. Slots are (qbp, h, kc) flattened.

    Units: ("q", sb, m), ("k", m, kb), ("v", kc), ("at", qc8, dt, qbp),
    ("pj", Et, qb). Placement rules keep each unit >= a few slots ahead
    of its first consumer (see design notes in the module docstring).
    """
    fillers = {}

    def put(qbp, h, kc, u):
        i = (qbp * HC + h) * nkc + min(kc, nkc - 1)
        fillers.setdefault(i, []).append(u)

    # v units, head-pair granular: pair p needed by pv(h=2p) which runs
    # during head 2p+1; spread them so no single head period overloads PE
    for kc in range(nkc):
        put(0, 0, kc, ("v", kc, 0))
        put(0, 2, kc, ("v", kc, 1))
        put(0, 4, kc, ("v", kc, 2))
    # k units (prologue does m0 kb0 only); m-tile m needed by heads 2m..;
    # kb block j only feeds score chunks kc >= 4j, so later blocks are JIT
    nkb = (nkc + 3) // 4
    for j in range(1, nkb):
        put(0, 0, 2 * j - 1, ("k", 0, j))
    put(0, 1, 0, ("k", 1, 0))
    put(0, 1, 6, ("k", 1, 1))
    put(0, 2, 1, ("k", 1, 2))
    put(0, 3, 5, ("k", 2, 0))
    put(0, 3, 7, ("k", 2, 1))
    put(0, 4, 1, ("k", 2, 2))
    # q units (prologue does sb0/sb1 m0); m-tile m needed by heads 2m
    put(0, 1, 2, ("q", 0, 1))
    put(0, 1, 4, ("q", 1, 1))
    put(0, 3, 1, ("q", 0, 2))
    put(0, 3, 3, ("q", 1, 2))
    put(0, 5, 1, ("q", 2, 0))
    put(0, 5, 3, ("q", 3, 0))
    put(1, 0, 1, ("q", 2, 1))
    put(1, 0, 3, ("q", 3, 1))
    put(1, 1, 1, ("q", 2, 2))
    put(1, 1, 3, ("q", 3, 2))
    # attT transposes: (qc8, dt) one head-period after norm(2dt+1, qc8).
    # dt1 of the second query half moves INTO (1,4): its pv-pair norms
    # land at slot qc8//2 there, and (1,5) must keep the un ring free for
    # the tracking accumulators (and its sc ring free for scores).
    # dt0/dt1 of the first query half run in (1,0)'s ACT-paced slack --
    # (0,3)/(0,5) are PE-overloaded; their proj consumers start at (1,1)
    for qc8 in range(8):
        put(1, 0, qc8, ("at", qc8, 0, 0))
        put(0, 5, qc8, ("at", qc8, 1, 0))
        put(1, 1, qc8, ("at", qc8, 2, 0))
        put(1, 3, qc8, ("at", qc8, 0, 1))
        put(1, 4, qc8 // 2 + 1, ("at", qc8, 1, 1))
    # proj qb0/qb1 spread through the (light) qbp1 head periods
    for Et in range(6):
        if Et < 3:
            put(1, 1, 5 + Et, ("pj", Et, 0))
        else:
            put(1, 2, 2 * (Et - 3) + 1, ("pj", Et, 0))
        if Et < 4:
            put(1, 3, 2 * Et + 1, ("pj", Et, 1))
        else:
            put(1, 4, 2 * (Et - 4) + 1, ("pj", Et, 1))
    return fillers


def _emit(nc, tc, nkc, xt_d, xtk_d, wq_d, wk_d, wv_d, wp_d, bq_d, bk_d,
          ones_d, y_d):
    kp = nkc * 128
    nkb = (kp + 511) // 512  # k-unit key blocks (512-wide, last ragged)
    ctx_pools = []

    def pool(name, bufs, space="SBUF"):
        p = tc.tile_pool(name=name, bufs=bufs, space=space)
        ctx_pools.append(p)
        return p.__enter__()

    consts = pool("consts", 1)
    store = pool("store", 1)
    pt_p = pool("pt", 2)
    sc_p = pool("sc", 2, space="PSUM")    # [128,1024] f32 = 2 banks each
    pv_p = pool("pv", 2, space="PSUM")    # [128,512] f32 = 1 bank each
    un_p = pool("un", 2, space="PSUM")    # [128,512] f32 = 1 bank each
    ys_p = pool("ys", 4)
    rs_p = pool("rs", 2)

    ident = consts.tile([128, 128], FP16)
    wq = consts.tile([128, KCH, 3, 128], FP16)
    wk = consts.tile([128, KCH, 3, 128], FP16)
    wv = consts.tile([128, KCH, VC], FP16)
    wp = consts.tile([128, 3, E], FP16)
    bq = consts.tile([128, 3], F32)
    bk = consts.tile([128, 3], F32)
    ones = consts.tile([128, nkc, HC], FP16)

    # The modeled DMA bus is near serial and only per-queue FIFO order is
    # controllable (SWDGE desc-gen has no waits, so it races the bus), so
    # ALL input loads go on the sync queue in exact priority order: the
    # critical path to the first exp (wk m0, xtk c0, wq m0, xt sb0/sb1)
    # first, then everything else by first use.


    xts = store.tile([128, KCH, S], FP16, name="xts")
    xtk = store.tile([128, KCH, kp], FP16, name="xtk")
    qT = [store.tile([128, 3, 512], FP16, name=f"qT{sb}") for sb in range(4)]
    kT = store.tile([128, 3, kp], FP16, name="kT")
    vst = store.tile([128, nkc, HC, VW], FP16, name="vst")
    att = store.tile([128, 16, VC], FP16, name="att")
    attT = store.tile([128, 3, S], FP16, name="attT")

    # x loads on the sync queue, halves first so q-unit matmuls can start
    # as soon as the first three contraction chunks land
    def load_xt(sb):
        for half in range(2):
            ks = slice(3 * half, 3 * half + 3)
            nc.sync.dma_start(
                xts[:, ks, sb * 512 : (sb + 1) * 512], xt_d.ap()[sb][:, ks, :]
            )

    def load_xtk(c0, c1):
        nc.sync.dma_start(xtk[:, :, c0:c1], xtk_d.ap()[:, :, c0:c1])

    nc.sync.dma_start(wk[:, :, 0, :], wk_d.ap()[:, :, 0, :])
    # first key block in contraction-halves so the first k-unit matmuls
    # start one transfer earlier
    nc.sync.dma_start(xtk[:, 0:3, 0:512], xtk_d.ap()[:, 0:3, 0:512])
    nc.sync.dma_start(xtk[:, 3:6, 0:512], xtk_d.ap()[:, 3:6, 0:512])
    nc.sync.dma_start(bk[:], bk_d.ap())
    nc.sync.dma_start(wq[:, :, 0, :], wq_d.ap()[:, :, 0, :])
    nc.sync.dma_start(bq[:], bq_d.ap())
    load_xt(0)
    load_xt(1)
    nc.sync.dma_start(ones[:], ones_d.ap())
    nc.sync.dma_start(wv[:], wv_d.ap())
    make_identity(nc, ident[:])
    if kp > 512:
        load_xtk(512, min(kp, 1024))
    nc.sync.dma_start(wq[:, :, 1:3, :], wq_d.ap()[:, :, 1:3, :])
    nc.sync.dma_start(wk[:, :, 1:3, :], wk_d.ap()[:, :, 1:3, :])
    if kp > 1024:
        load_xtk(1024, kp)
    load_xt(2)
    nc.sync.dma_start(wp[:], wp_d.ap())
    load_xt(3)

    # ---- units -----------------------------------------------------------
    def unit_q(sb, m):
        u = un_p.tile([128, 512], F32, tag="un", name=f"uq{sb}_{m}")
        for k in range(KCH):
            nc.tensor.matmul(
                u[:], wq[:, k, m, :], xts[:, k, sb * 512 : (sb + 1) * 512],
                start=(k == 0), stop=(k == KCH - 1),
            )
        nc.vector.tensor_scalar_add(qT[sb][:, m, :], u[:], bq[:, m : m + 1])

    def unit_k(m, kb):
        c0, c1 = kb * 512, min((kb + 1) * 512, kp)
        u = un_p.tile([128, 512], F32, tag="un", name=f"uk{m}_{kb}")
        for k in range(KCH):
            nc.tensor.matmul(
                u[:, 0 : c1 - c0], wk[:, k, m, :], xtk[:, k, c0:c1],
                start=(k == 0), stop=(k == KCH - 1),
            )
        nc.vector.tensor_scalar_add(
            kT[:, m, c0:c1], u[:, 0 : c1 - c0], bk[:, m : m + 1]
        )

    def unit_v(kc, p):
        # one head-pair's v columns: keeps the v work out of the first
        # head period (pv of head h only needs pair h//2's columns)
        u = un_p.tile([128, 512], F32, tag="un", name=f"uv{kc}_{p}")
        for k in range(KCH):
            nc.tensor.matmul(
                u[:, 0:128], xtk[:, k, kc * 128 : (kc + 1) * 128],
                wv[:, k, p * 128 : (p + 1) * 128],
                start=(k == 0), stop=(k == KCH - 1),
            )
        nc.vector.tensor_copy(
            vst[:, kc, 2 * p : 2 * p + 2, 0:D],
            u[:, 0:128].rearrange("p (h d) -> p h d", h=2),
        )
        nc.vector.tensor_copy(
            vst[:, kc, 2 * p : 2 * p + 2, D : D + 1],
            ones[:, kc : kc + 1, 2 * p : 2 * p + 2].rearrange(
                "p one h -> p h one"
            ),
        )

    def unit_attT(qc8, dt, qbp, pool=None, dma=False):
        qc = qbp * 8 + qc8
        if dma:
            # main-body transposes ride the DMA crossbar (8 xbar tiles,
            # ~112ns engine time) -- they have a full head period of slack,
            # and this frees PE rows and DVE copies in loaded periods
            nc.sync.dma_start_transpose(
                attT[:, dt, qc * 128 : (qc + 1) * 128],
                att[:, qc, dt * 128 : (dt + 1) * 128],
            )
            return
        tr = (pool or un_p).tile([128, 128], FP16,
                                 tag="sc" if pool is sc_p else "un",
                                 name=f"tr{qc}_{dt}")
        nc.tensor.matmul(
            tr[:], att[:, qc, dt * 128 : (dt + 1) * 128], ident[:],
            is_transpose=True, start=True, stop=True,
        )
        nc.vector.tensor_copy(attT[:, dt, qc * 128 : (qc + 1) * 128], tr[:])

    def unit_proj(Et, qb, pool=None, evac=None):
        u = (pool or un_p).tile([128, 512], F32,
                                tag="pv" if pool is pv_p else "un",
                                name=f"up{Et}_{qb}")
        for dt in range(3):
            nc.tensor.matmul(
                u[:], wp[:, dt, Et * 128 : (Et + 1) * 128],
                attT[:, dt, qb * 512 : (qb + 1) * 512],
                start=(dt == 0), stop=(dt == 2),
            )
        ys = ys_p.tile([128, 512], FP16, tag="ys", name="ys")
        if evac is nc.scalar:
            nc.scalar.copy(ys[:], u[:])
            # keep the y-store config off the ACT SEQ (it would serialize
            # with the evacuation copies)
            nc.sync.dma_start(y_d.ap()[Et][:, qb * 512 : (qb + 1) * 512],
                              ys[:])
        else:
            nc.vector.tensor_copy(ys[:], u[:])
            eng = nc.sync if (Et + qb) % 2 == 0 else nc.scalar
            eng.dma_start(y_d.ap()[Et][:, qb * 512 : (qb + 1) * 512], ys[:])

    def pv_mms(acc, pt, h, qcs, kcs, start, stop):
        n = len(qcs)
        for ki, kc in enumerate(kcs):
            for x, qc in enumerate(qcs):
                nc.tensor.matmul(
                    acc[:, x * VW : (x + 1) * VW],
                    pt[:, kc, (qc % 8) * 128 : (qc % 8 + 1) * 128],
                    vst[:, kc, h, :],
                    start=(start and ki == 0 and x == 0),
                    stop=(stop and ki == len(kcs) - 1 and x == n - 1),
                )

    def pv_norms(acc, h, qcs):
        for x, qc in enumerate(qcs):
            rse = rs_p.tile([128, 1], F32, tag="rs", name="rse")
            with nc.allow_low_precision(reason="f32r is full width"):
                nc.vector.reciprocal(rse[:], acc[:, x * VW + D : x * VW + D + 1])
            nc.vector.tensor_scalar_mul(
                att[:, qc, h * D : (h + 1) * D],
                acc[:, x * VW : x * VW + D], rse[:],
            )

    def pv_group(pt, h, qcs):
        # one PSUM bank accumulates len(qcs) (<=2) query chunks: a single
        # start/stop accumulation group, halving pv ring turnover
        acc = pv_p.tile([128, 512], F32, tag="pv", name=f"pv{qcs[0]}_{h}")
        pv_mms(acc, pt, h, qcs, range(nkc), True, True)
        pv_norms(acc, h, qcs)

    def emit_unit(u):
        kind = u[0]
        if kind == "q":
            unit_q(u[1], u[2])
        elif kind == "k":
            unit_k(u[1], u[2])
        elif kind == "v":
            unit_v(u[1], u[2])
        elif kind == "at":
            unit_attT(u[1], u[2], u[3])
        elif kind == "pj":
            unit_proj(u[1], u[2])

    # ---- prologue units --------------------------------------------------
    unit_k(0, 0)
    unit_q(0, 0)

    fillers = _build_schedule(nkc)
    nslots = 2 * HC * nkc
    sc_pending = []  # score tiles awaiting their exp, FIFO

    def scores_mm(st, flat, j):
        qbp, rem = divmod(flat, HC * nkc)
        h, kc = divmod(rem, nkc)
        hp, r0 = h // 2, (h % 2) * 64
        sb = 2 * qbp + j
        nc.tensor.matmul(
            st[:, j * 512 : (j + 1) * 512],
            kT[r0 : r0 + 64, hp, kc * 128 : (kc + 1) * 128],
            qT[sb][r0 : r0 + 64, hp, :],
            start=True, stop=True,
        )

    def emit_scores(flat):
        st = sc_p.tile([128, 1024], F32, tag="sc", name="st")
        scores_mm(st, flat, 0)
        scores_mm(st, flat, 1)
        sc_pending.append(st)

    # front pipeline: the j0 halves of the first two score chunks depend
    # only on the sb0 query block (early on the DMA bus); q(1,0) and the
    # j1 halves follow once sb1 lands. Cross-engine deps are emission-
    # ordered, so this ordering is what lets ACT start ~2us earlier.
    if nkc >= 2:
        st0 = sc_p.tile([128, 1024], F32, tag="sc", name="st0")
        st1 = sc_p.tile([128, 1024], F32, tag="sc", name="st1")
        scores_mm(st0, 0, 0)
        scores_mm(st1, 1, 0)
    else:
        unit_q(1, 0)
        emit_scores(0)

    # ---- main loop: exp(i) -> scores(i+1) -> pv -> fillers ---------------
    pt_prev = None  # pt tile whose pv groups run during this head period
    pt_cur = None
    # last head: its pv pairs accumulate DURING its own slots ("tracking"),
    # using the idle un ring for pairs 0/1 and the pv ring (as its previous
    # user drains) for pairs 2/3 -- the tail then starts at norms directly
    track = [None] * 4
    # pair -> first emission slot; tracking runs ONE chunk behind the exp
    # stream (8 waiting matmuls would overflow the 4-deep bypass window
    # and stall PE in-order behind the in-flight exp)
    talloc = {0: 1, 1: 1, 2: 5, 3: 6}
    for qbp in range(2):
        for h in range(HC):
            tracking = qbp == 1 and h == HC - 1 and nkc >= 6
            pt_prev = pt_cur
            pt_cur = pt_p.tile([128, nkc, 1024], FP16, tag="pt",
                               name=f"pt{qbp}_{h}")
            for kc in range(nkc):
                i = (qbp * HC + h) * nkc + kc
                if i == 0 and nkc >= 2:
                    # first two slots: exp in 512-wide halves, j0 halves
                    # first -- the sb0 query block lands well before sb1 on
                    # the serialized DMA bus, so ACT starts ~4us earlier
                    for stx, kx in ((st0, 0), (st1, 1)):
                        nc.scalar.activation(
                            pt_cur[:, kx, 0:512], stx[:, 0:512], Act.Exp,
                            scale=0.125,
                        )
                    unit_q(1, 0)
                    scores_mm(st0, 0, 1)
                    scores_mm(st1, 1, 1)
                    emit_scores(2)
                    for stx, kx in ((st0, 0), (st1, 1)):
                        nc.scalar.activation(
                            pt_cur[:, kx, 512:1024], stx[:, 512:1024],
                            Act.Exp, scale=0.125,
                        )
                    if nkc >= 4:
                        emit_scores(3)
                    for u in fillers.get(0, ()):
                        emit_unit(u)
                    continue
                if i == 1 and nkc >= 2:
                    for u in fillers.get(1, ()):
                        emit_unit(u)
                    continue
                st = sc_pending.pop(0)
                nc.scalar.activation(pt_cur[:, kc, :], st[:], Act.Exp,
                                     scale=0.125)
                # two-slot lookahead: tile (i+2)%2 is freed by this exp,
                # and heavy filler slots can no longer delay the next-next
                # score chunk (ACT stays fed through PE-overloaded periods)
                la = i + 2 if nkc >= 4 else i + 1
                if la < nslots and (nkc < 4 or la >= 4):
                    emit_scores(la)
                if pt_prev is not None:
                    # pair j of the previous head runs at slot j,
                    # clamped into this head period
                    pvs = []
                    for j in range(4):
                        if min(j, nkc - 1) == kc:
                            pvs += [2 * j, 2 * j + 1]
                    ph = (h - 1) % HC
                    pqbp = qbp if h > 0 else qbp - 1
                    for x in range(0, len(pvs), 2):
                        pv_group(pt_prev, ph,
                                 [pqbp * 8 + q for q in pvs[x : x + 2]])
                if tracking:
                    for j in range(4):
                        k0 = talloc[j]
                        if kc < k0:
                            continue
                        qcs = [8 + 2 * j, 9 + 2 * j]
                        if kc == k0:
                            pl, tg = (un_p, "un") if j < 2 else (pv_p, "pv")
                            track[j] = pl.tile([128, 512], F32, tag=tg,
                                               name=f"tk{j}")
                            pv_mms(track[j], pt_cur, h, qcs,
                                   range(0, kc), True, False)
                        else:
                            pv_mms(track[j], pt_cur, h, qcs, [kc - 1],
                                   False, False)
                for u in fillers.get(i, ()):
                    if tracking and u[0] == "at":
                        unit_attT(u[1], u[2], u[3], pool=sc_p)
                    else:
                        emit_unit(u)

    # ---- tail: last head's pv + attT dt2 + proj qb2/qb3 ------------------
    # interleave so PE never sits on the pv->norm->attT DVE chains: each
    # attT lags its pv by one group, proj units weave between pv groups as
    # soon as their four attT columns are present.
    if nkc >= 6:
        # finish the tracked accumulators (last key chunk) right after the
        # final exp; the tail is then norms -> attT dt2 -> proj qb2/qb3,
        # with the proj units rotating over both freed PSUM rings
        for j in range(4):
            pv_mms(track[j], pt_cur, HC - 1, [8 + 2 * j, 9 + 2 * j],
                   [nkc - 1], False, True)
        pv_norms(track[0], HC - 1, [8, 9])
        pv_norms(track[1], HC - 1, [10, 11])
        for qc8 in range(4):
            unit_attT(qc8, 2, 1, pool=sc_p)
        pv_norms(track[2], HC - 1, [12, 13])
        pv_norms(track[3], HC - 1, [14, 15])
        ys2 = store.tile([128, HC, 512], FP16, name="ys2")

        def proj_qb2(Et, pl):
            u = pl.tile([128, 512], F32, tag="pv" if pl is pv_p else "un",
                        name=f"up{Et}_2")
            for dt in range(3):
                nc.tensor.matmul(
                    u[:], wp[:, dt, Et * 128 : (Et + 1) * 128],
                    attT[:, dt, 2 * 512 : 3 * 512],
                    start=(dt == 0), stop=(dt == 2),
                )
            if Et % 2 == 0:
                nc.scalar.copy(ys2[:, Et, :], u[:])
            else:
                nc.vector.tensor_copy(ys2[:, Et, :], u[:])
            if Et == 2:
                nc.sync.dma_start(
                    y_d.ap()[0:3, :, 2 * 512 : 3 * 512].rearrange(
                        "e p s -> p e s"
                    ),
                    ys2[:, 0:3, :],
                )
            elif Et == 5:
                nc.sync.dma_start(
                    y_d.ap()[3:6, :, 2 * 512 : 3 * 512].rearrange(
                        "e p s -> p e s"
                    ),
                    ys2[:, 3:6, :],
                )

        proj_qb2(0, un_p)
        proj_qb2(1, un_p)
        for qc8 in range(4, 8):
            unit_attT(qc8, 2, 1, pool=sc_p)
        for Et, pl in ((2, pv_p), (3, pv_p), (4, un_p), (5, un_p)):
            proj_qb2(Et, pl)
        # final query block: evacuate into one staging tile and store in
        # two 3-tile DMAs -- per-store HWDGE configs would serialize the
        # end-of-kernel chain
        ys6 = store.tile([128, HC, 512], FP16, name="ys6")
        for Et, pl in ((0, pv_p), (1, pv_p), (2, un_p), (3, un_p),
                       (4, pv_p), (5, pv_p)):
            u = pl.tile([128, 512], F32, tag="pv" if pl is pv_p else "un",
                        name=f"up{Et}_3")
            for dt in range(3):
                nc.tensor.matmul(
                    u[:], wp[:, dt, Et * 128 : (Et + 1) * 128],
                    attT[:, dt, 3 * 512 : 4 * 512],
                    start=(dt == 0), stop=(dt == 2),
                )
            if Et % 2 == 0:
                nc.scalar.copy(ys6[:, Et, :], u[:])
            else:
                nc.vector.tensor_copy(ys6[:, Et, :], u[:])
            if Et == 2:
                nc.sync.dma_start(
                    y_d.ap()[0:3, :, 3 * 512 : 4 * 512].rearrange(
                        "e p s -> p e s"
                    ),
                    ys6[:, 0:3, :],
                )
            elif Et == 4:
                nc.sync.dma_start(
                    y_d.ap()[3:5, :, 3 * 512 : 4 * 512].rearrange(
                        "e p s -> p e s"
                    ),
                    ys6[:, 3:5, :],
                )
        # the very last store is a single tile so the end-of-kernel chain
        # rides the shortest possible transfer
        nc.sync.dma_start(
            y_d.ap()[5:6, :, 3 * 512 : 4 * 512].rearrange("e p s -> p e s"),
            ys6[:, 5:6, :],
        )
    else:
        for pp in range(4):
            pv_group(pt_cur, HC - 1, [8 + 2 * pp, 9 + 2 * pp])
            if pp >= 1:
                unit_attT(2 * pp - 2, 2, 1)
                unit_attT(2 * pp - 1, 2, 1)
        unit_attT(6, 2, 1)
        unit_attT(7, 2, 1)
        for Et in range(6):
            unit_proj(Et, 2)
        for Et in range(6):
            unit_proj(Et, 3)

    for p in reversed(ctx_pools):
        p.__exit__(None, None, None)


def make_core_inputs(x, mask, Wqkv, bqkv, Wproj, kp):
    """Slice full inputs into 8 per-core input maps (host-side layouts)."""
    x = np.asarray(x, np.float32)
    mask = np.asarray(mask)
    Wqkv = np.asarray(Wqkv, np.float32)
    bqkv = np.asarray(bqkv, np.float32)
    Wproj = np.asarray(Wproj, np.float32)
    nkc = kp // 128
    f16 = np.float16
    maps = []
    for c in range(8):
        b, hg = c // 2, c % 2
        h0 = hg * HC
        keep = np.nonzero(mask[b, 0, 0, :] != 0)[0]
        kept = len(keep)
        xt = x[b].T.reshape(KCH, 128, S).transpose(1, 0, 2)  # [p, kch, s]
        xt4 = np.ascontiguousarray(
            xt.reshape(128, KCH, 4, 512).transpose(2, 0, 1, 3).astype(f16)
        )
        xk = np.zeros((kp, E), np.float32)
        xk[:kept] = x[b, keep, :]
        xtk = np.ascontiguousarray(
            xk.T.reshape(KCH, 128, kp).transpose(1, 0, 2).astype(f16)
        )
        wq = Wqkv[:, h0 * D : (h0 + HC) * D]
        wq = np.ascontiguousarray(
            wq.reshape(KCH, 128, 3, 128).transpose(1, 0, 2, 3).astype(f16)
        )
        wkk = Wqkv[:, E + h0 * D : E + (h0 + HC) * D]
        wkk = np.ascontiguousarray(
            wkk.reshape(KCH, 128, 3, 128).transpose(1, 0, 2, 3).astype(f16)
        )
        wvv = Wqkv[:, 2 * E + h0 * D : 2 * E + (h0 + HC) * D]
        wvv = np.ascontiguousarray(
            wvv.reshape(KCH, 128, VC).transpose(1, 0, 2).astype(f16)
        )
        wpp = Wproj[hg * VC : (hg + 1) * VC, :]
        wpp = np.ascontiguousarray(
            wpp.reshape(3, 128, E).transpose(1, 0, 2).astype(f16)
        )
        bqq = np.ascontiguousarray(
            bqkv[h0 * D : (h0 + HC) * D].reshape(3, 128).T.astype(np.float32)
        )
        bkk = np.ascontiguousarray(
            bqkv[E + h0 * D : E + (h0 + HC) * D]
            .reshape(3, 128).T.astype(np.float32)
        )
        keepmask = (np.arange(kp) < kept).astype(f16).reshape(nkc, 128).T
        onesr = np.ascontiguousarray(
            np.repeat(keepmask[:, :, None], HC, axis=2).astype(f16)
        )
        maps.append(
            {
                "xt": xt4, "xtk": xtk, "wq": wq, "wk": wkk, "wv": wvv,
                "wp": wpp, "bq": bqq, "bk": bkk, "ones": onesr,
            }
        )
    return maps


def run(x, mask, Wqkv, bqkv, Wproj, bproj, trace=False, trace_cores=None):
    mask = np.asarray(mask)
    Wproj_np = np.asarray(Wproj, np.float32)
    bproj_np = np.asarray(bproj, np.float32)
    bqkv_np = np.asarray(bqkv, np.float32)
    kept = (mask[:, 0, 0, :] != 0).sum(axis=1)
    kp = max(128, int(-(-kept.max() // 128)) * 128)
    in_maps = make_core_inputs(x, mask, Wqkv, bqkv_np, Wproj_np, kp)

    nc = build_program(kp)
    try:
        res = run_bass_kernel_spmd(
            nc, in_maps, core_ids=list(range(8)), trace=trace,
            trace_cores=trace_cores,
        )
    except Exception:
        # transient device wedge -- one retry is usually enough
        res = run_bass_kernel_spmd(
            nc, in_maps, core_ids=list(range(8)), trace=trace,
            trace_cores=trace_cores,
        )

    # host-folded bias: v-bias passes through softmax (weights sum to 1)
    bv = bqkv_np[2 * E : 3 * E]
    bias_row = bv @ Wproj_np + bproj_np
    y = np.empty((B, S, E), np.float32)
    for b in range(B):
        p0 = res.results[2 * b]["y"].reshape(E, S).astype(np.float32)
        p1 = res.results[2 * b + 1]["y"].reshape(E, S).astype(np.float32)
        y[b] = p0.T + p1.T + bias_row
    return y, res


def kernel(x, mask, Wqkv, bqkv, Wproj, bproj):
    y, _ = run(x, mask, Wqkv, bqkv, Wproj, bproj, trace=False)
    return y
